# revision 1
# baseline (speedup 1.0000x reference)
"""BoostedCausalAttention on 8 trn2 NeuronCores — bf16 build.

Sharding: core c -> (batch b = c//4, head-group g = c%4, 4 heads each).
Within a 4-core batch group (Megatron-style):
  - qkv projections + attention computed per head-group in "transposed"
    layout (feature on partitions, token on free axis), bf16 matmuls with
    f32 PSUM accumulation.
  - Attention runs query-block-major so finished token halves ship early:
    pred^T goes out via two per-token-half bf16 AllGathers (residual path
    + gate strip); corr^T via a single 8-core bf16 AllToAll that delivers
    each core exactly its 512-token strip (cross-batch chunks are ignored).
  - Final output: each core emits y[512 tokens, 1024] f32 (pre-bias);
    host concatenates and adds bo.
"""

from contextlib import ExitStack

import numpy as np

import concourse.bass as bass
import concourse.bacc as bacc
import concourse.tile as tile
import concourse.mybir as mybir
from concourse import bass_utils

B, T, D = 2, 2048, 1024
H, DH = 16, 64
SCALE = DH ** -0.5
G = 4            # head groups (cores per batch)
HG = H // G      # heads per core = 4
CP = HG * DH     # channels per core = 256
TS = T // G      # token slice per core for gate/output phase = 512
KC = D // 128    # contraction chunks over D = 8
TH = T // 2      # token half = 1024

F32 = mybir.dt.float32
BF16 = mybir.dt.bfloat16
EXP = mybir.ActivationFunctionType.Exp
SIG = mybir.ActivationFunctionType.Sigmoid

GROUPS = [[0, 1, 2, 3], [4, 5, 6, 7]]
GROUP8 = [[0, 1, 2, 3, 4, 5, 6, 7]]


def _build(sim=False, reps=1):
    nc = bacc.Bacc("TRN2", target_bir_lowering=False, debug=False, num_devices=8)

    xT = nc.dram_tensor("xT", [D, T], BF16, kind="ExternalInput")
    wqk0 = nc.dram_tensor("wqk0", [D, 2 * CP], BF16, kind="ExternalInput")
    wv0 = nc.dram_tensor("wv0", [D, CP], BF16, kind="ExternalInput")
    bqk0 = nc.dram_tensor("bqk0", [128, 4], F32, kind="ExternalInput")
    bv0 = nc.dram_tensor("bv0", [128, 2], F32, kind="ExternalInput")
    wqk1 = nc.dram_tensor("wqk1", [D, 2 * CP], BF16, kind="ExternalInput")
    wv1 = nc.dram_tensor("wv1", [D, CP], BF16, kind="ExternalInput")
    bqk1 = nc.dram_tensor("bqk1", [128, 4], F32, kind="ExternalInput")
    bv1 = nc.dram_tensor("bv1", [128, 2], F32, kind="ExternalInput")
    wg = nc.dram_tensor("wg", [2 * D, D], BF16, kind="ExternalInput")
    bg = nc.dram_tensor("bg", [128, D // 128], F32, kind="ExternalInput")
    wo = nc.dram_tensor("wo", [D, D], BF16, kind="ExternalInput")
    mask01 = nc.dram_tensor("mask01", [128, 128], BF16, kind="ExternalInput")
    onesc = nc.dram_tensor("onesc", [128, HG], BF16, kind="ExternalInput")
    ones64 = nc.dram_tensor("ones64", [1, 64], BF16, kind="ExternalInput")
    y = nc.dram_tensor("y", [TS, D], F32, kind="ExternalOutput")

    with tile.TileContext(nc) as tc, ExitStack() as glb:
        consts = glb.enter_context(tc.tile_pool(name="consts", bufs=1))
        # 4KB-per-partition slots: x^T/residual tiles, later reused by Wg
        big8 = glb.enter_context(tc.tile_pool(name="big8", bufs=8))
        dpool = glb.enter_context(tc.tile_pool(name="dpool", bufs=1, space="DRAM"))

        # ---- constants ----
        bqk_sb = [consts.tile([128, 4], F32, name=f"bqk_sb{r}") for r in range(2)]
        bv_sb = [consts.tile([128, 2], F32, name=f"bv_sb{r}") for r in range(2)]
        bg_sb = consts.tile([128, D // 128], F32)
        mask_sb = consts.tile([128, 128], BF16)
        ones_sb = consts.tile([128, HG], BF16)
        ones64_sb = consts.tile([1, 64], BF16)
        nc.gpsimd.dma_start(ones_sb[:], onesc[:, :])
        nc.gpsimd.dma_start(ones64_sb[:], ones64[:, :])
        nc.gpsimd.dma_start(bqk_sb[0][:], bqk0[:, :])
        nc.gpsimd.dma_start(bqk_sb[1][:], bqk1[:, :])
        nc.gpsimd.dma_start(bv_sb[0][:], bv0[:, :])
        nc.gpsimd.dma_start(bv_sb[1][:], bv1[:, :])
        nc.gpsimd.dma_start(bg_sb[:], bg[:, :])
        nc.gpsimd.dma_start(mask_sb[:], mask01[:, :])

        # ---- collective DRAM buffers ----
        # pred AG, split by token half (contiguous per-half tensors)
        pred_part = [dpool.tile([CP, TH], BF16, name=f"pred_part{i}")
                     for i in range(2)]
        pred_half = [dpool.tile([D, TH], BF16, name=f"pred_half{i}")
                     for i in range(2)]
        # pred strips for the gate phase ride a separate 8-core AllToAll
        # hidden under attention 1
        pred_a2a = dpool.tile([8 * CP, TS], BF16, name="pred_a2a")
        pred_sout = dpool.tile([8 * CP, TS], BF16, name="pred_sout")
        # corr A2A, split by head pair (8 chunks x 128 rows each)
        corr_a2a = [dpool.tile([8 * 128, TS], BF16, name=f"corr_a2a{i}")
                    for i in range(2)]
        corr_out = [dpool.tile([8 * 128, TS], BF16, name=f"corr_out{i}")
                    for i in range(2)]

        wqk_pool = glb.enter_context(tc.tile_pool(name="wqk", bufs=KC))
        qk_pool = glb.enter_context(tc.tile_pool(name="qk", bufs=8))
        wo_pool = glb.enter_context(tc.tile_pool(name="wo", bufs=KC))

        # strip-select registers are per-engine (snap donates in place);
        # set up once, reused by every rep
        def make_sel(eng, div):
            pid = eng.partition_id()
            reg = eng.alloc_register("sel")
            if div:
                eng.reg_div(reg, pid, G)
            else:
                eng.reg_mod(reg, pid, G)
            return eng.snap(reg, donate=True, min_val=0, max_val=1 if div
                            else G - 1)

        bsel_g = make_sel(nc.gpsimd, div=True)    # batch index b = pid//4
        bsel_y = make_sel(nc.sync, div=True)

        _body(nc, tc, sim, reps, big8, wqk_pool, qk_pool, wo_pool, bqk_sb, bv_sb, bg_sb, mask_sb, ones_sb, ones64_sb, bsel_g, bsel_y, xT, wqk0, wv0, wqk1, wv1, wg, wo, y, pred_part, pred_half, pred_a2a, pred_sout, corr_a2a, corr_out)

    nc.compile()
    return nc


def _body(nc, tc, sim, reps, big8, wqk_pool, qk_pool, wo_pool, bqk_sb, bv_sb, bg_sb, mask_sb, ones_sb, ones64_sb, bsel_g, bsel_y, xT, wqk0, wv0, wqk1, wv1, wg, wo, y, pred_part, pred_half, pred_a2a, pred_sout, corr_a2a, corr_out):
    for _rep in range(reps):
        # x^T resident tiles, reallocated per rep (slots cycle via the pool)
        xt = []
        for kc in range(KC):
            xt.append(big8.tile([128, T], BF16, name=f"xt{kc}", tag="b8"))
        with ExitStack() as att:
            wv_pool = att.enter_context(tc.tile_pool(name="wv", bufs=KC))
            vaug_pool = att.enter_context(tc.tile_pool(name="vaug", bufs=24))
            p_pool = att.enter_context(tc.tile_pool(name="pp", bufs=8))
            o_pool = att.enter_context(tc.tile_pool(name="op", bufs=6))
            bc_pool = att.enter_context(tc.tile_pool(name="bc", bufs=4))
            sm_pool = att.enter_context(tc.tile_pool(name="sm", bufs=4))
            psum = att.enter_context(tc.tile_pool(name="psum", bufs=3, space="PSUM"))
            avps = att.enter_context(tc.tile_pool(name="avps", bufs=5, space="PSUM"))

            def load_wqk(wqk_d, rnd):
                wt = []
                for kc in range(KC):
                    t_ = wqk_pool.tile([128, 2 * CP], BF16,
                                       name=f"wqk{rnd}_{kc}", tag="wqk")
                    eng = nc.scalar if kc % 2 else nc.sync
                    eng.dma_start(t_[:], wqk_d[128 * kc:128 * (kc + 1), :])
                    wt.append(t_)
                return wt

            def load_wv(wv_d, rnd):
                wt = []
                for kc in range(KC):
                    t_ = wv_pool.tile([128, CP], BF16,
                                      name=f"wv{rnd}_{kc}", tag="wv")
                    eng = nc.scalar if kc % 2 else nc.sync
                    eng.dma_start(t_[:], wv_d[128 * kc:128 * (kc + 1), :])
                    wt.append(t_)
                return wt

            def proj_qk(wt, src, biasc, rnd):
                """q^T|k^T [512 rows, T] as 4 tiles [128, T] (0-1: q, 2-3: k).
                Token-half-major so each half unblocks on its AG."""
                qk = []
                for jc in range(4):
                    qk.append(qk_pool.tile([128, T], BF16,
                                           name=f"qk{rnd}_{jc}", tag="qk"))
                for t4 in range(4):
                    for jc in range(4):
                        ps = psum.tile([128, 512], F32, tag="ps", name="ps_pqk")
                        for kc in range(KC):
                            nc.tensor.matmul(
                                ps[:], wt[kc][:, 128 * jc:128 * (jc + 1)],
                                src[kc][:, 512 * t4:512 * (t4 + 1)],
                                start=(kc == 0), stop=(kc == KC - 1))
                        nc.vector.tensor_scalar_add(
                            qk[jc][:, 512 * t4:512 * (t4 + 1)], ps[:],
                            biasc[:, jc:jc + 1])
                return qk

            def proj_v(wt, src, rnd):
                """v in natural layout + ones col: 16 tiles [128, HG, DH+1]."""
                va = []
                for tb in range(16):
                    t_ = vaug_pool.tile([128, HG, DH + 1], BF16,
                                        name=f"va{rnd}_{tb}", tag="va")
                    ps = psum.tile([128, 512], F32, tag="ps", name="ps_pv")
                    for kc in range(KC):
                        nc.tensor.matmul(
                            ps[:, 0:CP], src[kc][:, 128 * tb:128 * (tb + 1)],
                            wt[kc][:], start=(kc == 0), stop=(kc == KC - 1))
                    nc.vector.tensor_copy(
                        t_[:, :, 0:DH],
                        ps[:, 0:CP].rearrange("p (h d) -> p h d", h=HG))
                    nc.vector.tensor_copy(t_[:, :, DH:DH + 1],
                                          ones_sb[:, :, None])
                    va.append(t_)
                return va

            def attend(qk, va, biasv, emit, order):
                """Causal attention for 4 heads (2 pairs sharing PE row
                groups). order="q4": query-block-major, emit(half, ob) ships
                each finished token half of both pairs (early AllGather).
                order="hh": head-pair-major, emit(hh, ob) ships each pair's
                full output as it completes (early AllToAll half)."""
                ob = [o_pool.tile([128, T], BF16, tag="o", name=f"ob{hh}")
                      for hh in range(2)]
                loop = ([(q4, hh) for q4 in range(4) for hh in range(2)]
                        if order == "q4" else
                        [(q4, hh) for hh in range(2) for q4 in range(4)])
                for q4, hh in loop:
                    nblk = 4 * (q4 + 1)
                    if True:
                        av = [avps.tile([DH + 1, 512], F32, tag="av",
                                        name=f"av{h2}") for h2 in range(2)]
                        for kb in range(nblk):
                            for h2 in range(2):
                                h = 2 * hh + h2
                                base = 64 * h2
                                qt, kt = qk[hh], qk[2 + hh]
                                diag = kb - 4 * q4
                                c0 = max(0, 128 * diag)
                                npr = 512 - c0
                                ps = psum.tile([128, 512], F32, tag="ps",
                                               name=f"s{h2}")
                                nc.tensor.matmul(
                                    ps[:, 0:npr],
                                    kt[base:base + 64,
                                       128 * kb:128 * (kb + 1)],
                                    qt[base:base + 64,
                                       512 * q4 + c0:512 * (q4 + 1)],
                                    start=True, stop=True)
                                p = p_pool.tile([128, 512], BF16, tag="p",
                                                name=f"p{h2}")
                                nc.scalar.activation(p[:, 0:npr], ps[:, 0:npr],
                                                     EXP, scale=SCALE)
                                if diag >= 0:
                                    # zero the future-token triangle of the
                                    # diagonal block (replaces -inf mask add)
                                    nc.vector.tensor_mul(
                                        p[:, 0:128], p[:, 0:128], mask_sb[:])
                                nc.tensor.matmul(
                                    av[h2][:, c0:512], va[kb][:, h, :],
                                    p[:, 0:npr],
                                    start=(kb == 0), stop=(kb == nblk - 1))
                        recr = sm_pool.tile([1, 1024], BF16, tag="recr",
                                            name="recr")
                        with nc.allow_low_precision(
                                reason="softmax recip rounds to bf16"):
                            nc.vector.reciprocal(recr[0:1, 0:512],
                                                 av[0][DH:DH + 1, :])
                            nc.vector.reciprocal(recr[0:1, 512:1024],
                                                 av[1][DH:DH + 1, :])
                        bps = avps.tile([128, 512], F32, tag="av", name="bps")
                        for h2 in range(2):
                            nc.tensor.matmul(
                                bps[64 * h2:64 * (h2 + 1), :], ones64_sb[:],
                                recr[0:1, 512 * h2:512 * (h2 + 1)],
                                start=True, stop=True)
                        bc = bc_pool.tile([128, 512], F32, tag="bc", name="bc")
                        nc.vector.tensor_copy(bc[:], bps[:])
                        osl = ob[hh][:, 512 * q4:512 * (q4 + 1)]
                        for h2 in range(2):
                            nc.vector.tensor_mul(
                                osl[64 * h2:64 * (h2 + 1), :],
                                av[h2][0:DH, :],
                                bc[64 * h2:64 * (h2 + 1), :])
                        nc.vector.tensor_scalar_add(osl, osl,
                                                    biasv[:, hh:hh + 1])
                    if order == "q4" and hh == 1 and q4 % 2 == 1:
                        emit(q4 // 2, ob)
                    elif order == "hh" and q4 == 3:
                        emit(hh, ob[hh])

            # ================= round 0 =================
            wv0_t = load_wv(wv0, 0)
            for hf in range(4):
                for kc in range(KC):
                    eng = nc.scalar if (kc + hf) % 2 else nc.sync
                    eng.dma_start(xt[kc][:, 512 * hf:512 * (hf + 1)],
                                  xT[128 * kc:128 * (kc + 1),
                                     512 * hf:512 * (hf + 1)])
                if hf == 0:
                    wqk0_t = load_wqk(wqk0, 0)
            va0 = proj_v(wv0_t, xt, 0)
            qk0 = proj_qk(wqk0_t, xt, bqk_sb[0], 0)

            def emit_pred(half, ob):
                cs = slice(TH * half, TH * (half + 1))
                for hh in range(2):
                    nc.sync.dma_start(
                        pred_part[half][128 * hh:128 * (hh + 1), :],
                        ob[hh][:, cs])
                for s in (2 * half, 2 * half + 1):
                    for dup in range(2):
                        c_ = 4 * dup + s
                        for hh in range(2):
                            r0 = CP * c_ + 128 * hh
                            nc.sync.dma_start(
                                pred_a2a[r0:r0 + 128, :],
                                ob[hh][:, 512 * s:512 * (s + 1)])
                if sim:
                    nc.sync.dma_start(pred_half[half][0:CP, :],
                                      pred_part[half][:, :])
                    if half == 1:
                        nc.sync.dma_start(pred_sout[0:128, 0:64],
                                          pred_a2a[0:128, 0:64])
                else:
                    nc.gpsimd.collective_compute(
                        "AllGather", mybir.AluOpType.bypass,
                        replica_groups=GROUPS,
                        ins=[pred_part[half][:, :]],
                        outs=[pred_half[half][:, :]])
                    if half == 1:
                        nc.gpsimd.collective_compute(
                            "AllToAll", mybir.AluOpType.bypass,
                            replica_groups=GROUP8,
                            ins=[pred_a2a[:, :]],
                            outs=[pred_sout[:, :]])

            attend(qk0, va0, bv_sb[0], emit_pred, order="q4")

            # residual in place: xt <- xt - pred^T, token-half-major so
            # the t4-major round-1 projection unblocks as each AG half lands
            for hf in range(2):
                cs = slice(TH * hf, TH * (hf + 1))
                for kc in range(KC):
                    pt = o_pool.tile([128, TH], BF16, tag="o", name="predld")
                    nc.sync.dma_start(
                        pt[:], pred_half[hf][128 * kc:128 * (kc + 1), :])
                    nc.vector.tensor_sub(xt[kc][:, cs], xt[kc][:, cs], pt[:])

            # ================= round 1 =================
            wv1_t = load_wv(wv1, 1)
            wqk1_t = load_wqk(wqk1, 1)
            va1 = proj_v(wv1_t, xt, 1)
            qk1 = proj_qk(wqk1_t, xt, bqk_sb[1], 1)

            # prefetch gate-phase pred strips into the wqk slots freed by
            # the round-1 projection (overlaps attention 1); A2A output rows
            # are member-major, so my batch's rows are in global channel order
            pred_bv = pred_sout[:, :].rearrange("(b r) t -> b r t", b=2)
            predg = []
            for cc in range(KC):
                pg_ = wqk_pool.tile([128, TS], BF16, name=f"predg{cc}",
                                    tag="wqk")
                nc.gpsimd.dma_start(
                    pg_[:], pred_bv[bass.ds(bsel_g, 1),
                                    128 * cc:128 * (cc + 1), :].squeeze(0))
                predg.append(pg_)

            # hoist gate/output weight loads ahead of the corr exchange in
            # their engines' program order: an in-order SEQ stalls on the
            # collective's input dep, so anything queued behind it would
            # only issue after attention 1 drains.
            wg_t = []
            for i in range(KC):
                t_ = big8.tile([128, 2 * D], BF16, name=f"wg{i}", tag="b8")
                nc.gpsimd.dma_start(
                    t_[:].rearrange("p (a d) -> p a d", a=2),
                    bass.AP(tensor=wg, offset=256 * i * D,
                            ap=[[D, 128], [128 * D, 2], [1, D]]))
                wg_t.append(t_)

            wo_t = []
            for cc in range(KC):
                t_ = wo_pool.tile([128, D], BF16, name=f"wo{cc}", tag="wo")
                nc.sync.dma_start(t_[:], wo[128 * cc:128 * (cc + 1), :])
                wo_t.append(t_)

            def emit_corr(hh, obh):
                # chunk for destination core c' = my 128 pair-hh channels of
                # that core's token strip; both batch groups get the same
                # strip payload (A2A is 8-wide, cross-batch chunks ignored)
                for s in range(G):
                    for dup in range(2):
                        c_ = 4 * dup + s
                        nc.sync.dma_start(
                            corr_a2a[hh][128 * c_:128 * (c_ + 1), :],
                            obh[:, 512 * s:512 * (s + 1)])
                if sim:
                    nc.sync.dma_start(corr_out[hh][0:128, 0:64],
                                      corr_a2a[hh][0:128, 0:64])
                else:
                    nc.gpsimd.collective_compute(
                        "AllToAll", mybir.AluOpType.bypass,
                        replica_groups=GROUP8,
                        ins=[corr_a2a[hh][:, :]],
                        outs=[corr_out[hh][:, :]])

            attend(qk1, va1, bv_sb[1], emit_corr, order="hh")

        # ================= gate + output ==================
        with ExitStack() as gat:
            gp_pool = gat.enter_context(tc.tile_pool(name="gp", bufs=8))
            y_pool = gat.enter_context(tc.tile_pool(name="yp", bufs=2))
            ps2 = gat.enter_context(tc.tile_pool(name="ps2", bufs=8,
                                                 space="PSUM"))

            def wg_slice(cc, jc):
                return wg_t[cc // 2][:, D * (cc % 2) + 128 * jc:
                                     D * (cc % 2) + 128 * (jc + 1)]

            # corr strip: channel 128cc belongs to group member cc//2,
            # head pair cc%2 — even chunks land with A2A half 0 (fires mid
            # attention 1), odd with half 1; consume in that order so the
            # tail AllToAll overlaps the gate matmul
            corr_bv = [t[:, :].rearrange("(b r) t -> b r t", b=2)
                       for t in corr_out]
            corr_t = [qk_pool.tile([128, T], BF16, name=f"corrt{i}", tag="qk")
                      for i in range(2)]
            for cc in [0, 2, 4, 6, 1, 3, 5, 7]:
                r0 = 128 * (cc // 2)
                nc.sync.dma_start(
                    corr_t[cc // 4][:, 512 * (cc % 4):512 * (cc % 4 + 1)],
                    corr_bv[cc % 2][bass.ds(bsel_y, 1),
                                    r0:r0 + 128, :].squeeze(0))
            corrg = [corr_t[cc // 4][:, 512 * (cc % 4):512 * (cc % 4 + 1)]
                     for cc in range(KC)]

            gps = []
            for jc in range(KC):
                ps = ps2.tile([128, 512], F32, tag="ps2", name=f"ps_g{jc}")
                for cc in range(KC):
                    nc.tensor.matmul(ps[:], wg_slice(cc, jc), predg[cc][:],
                                     start=(cc == 0), stop=False)
                gps.append(ps)
            pgt = []
            for jc in range(KC):
                ps = gps[jc]
                for i, cc in enumerate([0, 2, 4, 6, 1, 3, 5, 7]):
                    nc.tensor.matmul(ps[:], wg_slice(KC + cc, jc), corrg[cc],
                                     start=False, stop=(i == KC - 1))
                gt = gp_pool.tile([128, TS], BF16, name=f"gate{jc}", tag="gp")
                nc.scalar.activation(gt[:], ps[:], SIG, bias=bg_sb[:, jc:jc + 1])
                nc.vector.tensor_mul(gt[:], gt[:], corrg[jc])
                nc.vector.tensor_add(gt[:], gt[:], predg[jc][:])
                pgt.append(gt)

            for tb in range(4):
                yt = y_pool.tile([128, D], F32, tag="y", name="yt")
                for n2 in range(2):
                    ps = ps2.tile([128, 512], F32, tag="ps2", name="ps_y")
                    for cc in range(KC):
                        nc.tensor.matmul(
                            ps[:], pgt[cc][:, 128 * tb:128 * (tb + 1)],
                            wo_t[cc][:, 512 * n2:512 * (n2 + 1)],
                            start=(cc == 0), stop=(cc == KC - 1))
                    nc.vector.tensor_copy(yt[:, 512 * n2:512 * (n2 + 1)],
                                          ps[:])
                nc.sync.dma_start(y[128 * tb:128 * (tb + 1), :], yt[:])


_NC = None


def _get_nc():
    global _NC
    if _NC is None:
        _NC = _build()
    return _NC


def make_in_maps(x, Wqkv0, bqkv0, Wqkv1, bqkv1, Wg, bg, Wo, bo):
    bf = mybir.dt.np(BF16)
    mask_np = np.where(np.arange(128)[:, None] > np.arange(128)[None, :],
                       0.0, 1.0).astype(bf)
    ones_np = np.ones((128, HG), bf)
    ones64_np = np.ones((1, 64), bf)
    bg_a = np.ascontiguousarray(bg.reshape(D // 128, 128).T.astype(np.float32))
    wg_np = np.ascontiguousarray(Wg.astype(np.float32).astype(bf))
    wo_np = np.ascontiguousarray(Wo.astype(np.float32).astype(bf))

    in_maps = []
    for c in range(8):
        b, g = divmod(c, G)
        cq = slice(CP * g, CP * (g + 1))
        ck = slice(D + CP * g, D + CP * (g + 1))
        cv = slice(2 * D + CP * g, 2 * D + CP * (g + 1))
        m = {
            "xT": np.ascontiguousarray(x[b].T.astype(np.float32).astype(bf)),
            "mask01": mask_np, "onesc": ones_np, "bg": bg_a,
            "ones64": ones64_np,
            "wg": wg_np, "wo": wo_np,
        }
        for r, (W, bb) in enumerate(((Wqkv0, bqkv0), (Wqkv1, bqkv1))):
            m[f"wqk{r}"] = np.ascontiguousarray(
                np.concatenate([W[:, cq], W[:, ck]], axis=1)
                .astype(np.float32).astype(bf))
            m[f"wv{r}"] = np.ascontiguousarray(
                W[:, cv].astype(np.float32).astype(bf))
            bqk_cat = np.concatenate([bb[cq], bb[ck]]).astype(np.float32)
            m[f"bqk{r}"] = np.ascontiguousarray(bqk_cat.reshape(4, 128).T)
            m[f"bv{r}"] = np.ascontiguousarray(
                bb[cv].astype(np.float32).reshape(2, 128).T)
        in_maps.append(m)
    return in_maps


def assemble(results, bo):
    out = np.empty((B, T, D), np.float32)
    for c in range(8):
        b, g = divmod(c, G)
        out[b, TS * g:TS * (g + 1), :] = results[c]["y"]
    return out + bo.astype(np.float32)


def kernel(x, Wqkv0, bqkv0, Wqkv1, bqkv1, Wg, bg, Wo, bo):
    args = [np.asarray(a) for a in
            (x, Wqkv0, bqkv0, Wqkv1, bqkv1, Wg, bg, Wo, bo)]
    nc = _get_nc()
    in_maps = make_in_maps(*args)
    res = bass_utils.run_bass_kernel_spmd(nc, in_maps, core_ids=list(range(8)))
    return assemble(res.results, args[8])



# revision 2
# speedup vs baseline: 1.0587x; 1.0587x over previous
"""BoostedCausalAttention on 8 trn2 NeuronCores — software-pipelined bf16.

Sharding: core c -> (batch b = c//4, head-group g = c%4, 4 heads each).
Within a 4-core batch group (Megatron-style):
  - qkv projections + attention in "transposed" layout (feature on
    partitions, token on free axis), bf16 matmuls with f32 PSUM accum.
  - The attention inner loop is software-pipelined: the scores matmul for
    key block kb+1 is emitted BEFORE the AV matmul of block kb, so the PE
    never head-of-line blocks on the exp; exp covers both heads of a pair
    in one [128,2,npr] activation.
  - Cross-phase pipelining: round-1 projection interleaves into round-0
    attention's tail; the gate's pred-half matmuls interleave into
    round-1 attention; AllGathers/AllToAlls fire per token-half /
    head-pair so their latency hides under compute.
  - Final output: each core emits y[512 tokens, 1024] f32 (pre-bias);
    host concatenates and adds bo.
"""

from contextlib import ExitStack

import numpy as np

import concourse.bass as bass
import concourse.bacc as bacc
import concourse.tile as tile
import concourse.mybir as mybir
from concourse import bass_utils

B, T, D = 2, 2048, 1024
H, DH = 16, 64
SCALE = DH ** -0.5
G = 4            # head groups (cores per batch)
HG = H // G      # heads per core = 4
CP = HG * DH     # channels per core = 256
TS = T // G      # token slice per core for gate/output phase = 512
KC = D // 128    # contraction chunks over D = 8
TH = T // 2      # token half = 1024

F32 = mybir.dt.float32
BF16 = mybir.dt.bfloat16
EXP = mybir.ActivationFunctionType.Exp
SIG = mybir.ActivationFunctionType.Sigmoid

GROUPS = [[0, 1, 2, 3], [4, 5, 6, 7]]
GROUP8 = [[0, 1, 2, 3, 4, 5, 6, 7]]


def _build(sim=False, reps=1):
    nc = bacc.Bacc("TRN2", target_bir_lowering=False, debug=False, num_devices=8)

    xT = nc.dram_tensor("xT", [D, T], BF16, kind="ExternalInput")
    wqk0 = nc.dram_tensor("wqk0", [D, 2 * CP], BF16, kind="ExternalInput")
    wv0 = nc.dram_tensor("wv0", [D, CP], BF16, kind="ExternalInput")
    bqk0 = nc.dram_tensor("bqk0", [128, 4], F32, kind="ExternalInput")
    bv0 = nc.dram_tensor("bv0", [128, 2], F32, kind="ExternalInput")
    wqk1 = nc.dram_tensor("wqk1", [D, 2 * CP], BF16, kind="ExternalInput")
    wv1 = nc.dram_tensor("wv1", [D, CP], BF16, kind="ExternalInput")
    bqk1 = nc.dram_tensor("bqk1", [128, 4], F32, kind="ExternalInput")
    bv1 = nc.dram_tensor("bv1", [128, 2], F32, kind="ExternalInput")
    wg = nc.dram_tensor("wg", [2 * D, D], BF16, kind="ExternalInput")
    bg = nc.dram_tensor("bg", [128, D // 128], F32, kind="ExternalInput")
    wo = nc.dram_tensor("wo", [D, D], BF16, kind="ExternalInput")
    mask01 = nc.dram_tensor("mask01", [128, 128], BF16, kind="ExternalInput")
    onesc = nc.dram_tensor("onesc", [128, HG], BF16, kind="ExternalInput")
    ones64 = nc.dram_tensor("ones64", [1, 64], BF16, kind="ExternalInput")
    y = nc.dram_tensor("y", [TS, D], F32, kind="ExternalOutput")

    with tile.TileContext(nc) as tc, ExitStack() as glb:
        consts = glb.enter_context(tc.tile_pool(name="consts", bufs=1))
        # 4KB-per-partition slots: x^T/residual tiles, later reused by Wg
        big8 = glb.enter_context(tc.tile_pool(name="big8", bufs=8))
        dpool = glb.enter_context(tc.tile_pool(name="dpool", bufs=1, space="DRAM"))

        # ---- constants ----
        bqk_sb = [consts.tile([128, 4], F32, name=f"bqk_sb{r}") for r in range(2)]
        bv_sb = [consts.tile([128, 2], F32, name=f"bv_sb{r}") for r in range(2)]
        bg_sb = consts.tile([128, D // 128], F32)
        mask_sb = consts.tile([128, 128], BF16)
        ones_sb = consts.tile([128, HG], BF16)
        ones64_sb = consts.tile([1, 64], BF16)
        nc.gpsimd.dma_start(ones_sb[:], onesc[:, :])
        nc.gpsimd.dma_start(ones64_sb[:], ones64[:, :])
        nc.gpsimd.dma_start(bqk_sb[0][:], bqk0[:, :])
        nc.gpsimd.dma_start(bqk_sb[1][:], bqk1[:, :])
        nc.gpsimd.dma_start(bv_sb[0][:], bv0[:, :])
        nc.gpsimd.dma_start(bv_sb[1][:], bv1[:, :])
        nc.gpsimd.dma_start(bg_sb[:], bg[:, :])
        nc.gpsimd.dma_start(mask_sb[:], mask01[:, :])

        # ---- collective DRAM buffers ----
        pred_part = [dpool.tile([CP, TH], BF16, name=f"pred_part{i}")
                     for i in range(2)]
        pred_full = dpool.tile([2, D, TH], BF16, name="pred_full")
        corr_a2a = [dpool.tile([8 * 128, TS], BF16, name=f"corr_a2a{i}")
                    for i in range(2)]
        corr_out = [dpool.tile([8 * 128, TS], BF16, name=f"corr_out{i}")
                    for i in range(2)]

        wqk_pool = glb.enter_context(tc.tile_pool(name="wqk", bufs=KC))
        qk_pool = glb.enter_context(tc.tile_pool(name="qk", bufs=8))
        wo_pool = glb.enter_context(tc.tile_pool(name="wo", bufs=KC))
        gpre_pool = glb.enter_context(tc.tile_pool(name="gpre", bufs=KC))

        # strip-select registers (snap donates in place); set up once.
        # g = pid%4: half hf = g//2, within-half slot st = g%2.
        pid = nc.gpsimd.partition_id()
        r_t = nc.gpsimd.alloc_register("selt")
        nc.gpsimd.reg_div(r_t, pid, 2)
        half_sel = nc.gpsimd.snap(r_t, donate=True, min_val=0, max_val=3)
        r_hf = nc.gpsimd.alloc_register("selhf")
        nc.gpsimd.reg_mod(r_hf, half_sel, 2)
        hf_sel = nc.gpsimd.snap(r_hf, donate=True, min_val=0, max_val=1)
        r_st = nc.gpsimd.alloc_register("selst")
        nc.gpsimd.reg_mod(r_st, pid, 2)
        st_sel = nc.gpsimd.snap(r_st, donate=True, min_val=0, max_val=1)

        pid_y = nc.sync.partition_id()
        r_by = nc.sync.alloc_register("selby")
        nc.sync.reg_div(r_by, pid_y, G)
        bsel_y = nc.sync.snap(r_by, donate=True, min_val=0, max_val=1)

        _body(nc, tc, sim, reps, big8, wqk_pool, qk_pool, wo_pool, gpre_pool,
              bqk_sb, bv_sb, bg_sb, mask_sb, ones_sb, ones64_sb,
              hf_sel, st_sel, bsel_y, xT, wqk0, wv0, wqk1, wv1, wg, wo, y,
              pred_part, pred_full, corr_a2a, corr_out)

    nc.compile()
    return nc


def _body(nc, tc, sim, reps, big8, wqk_pool, qk_pool, wo_pool, gpre_pool,
          bqk_sb, bv_sb, bg_sb, mask_sb, ones_sb, ones64_sb,
          hf_sel, st_sel, bsel_y, xT, wqk0, wv0, wqk1, wv1, wg, wo, y,
          pred_part, pred_full, corr_a2a, corr_out):
    for _rep in range(reps):
        xt = []
        for kc in range(KC):
            xt.append(big8.tile([128, T], BF16, name=f"xt{kc}", tag="b8"))
        with ExitStack() as att:
            wv_pool = att.enter_context(tc.tile_pool(name="wv", bufs=KC))
            vaug_pool = att.enter_context(tc.tile_pool(name="vaug", bufs=24))
            p_pool = att.enter_context(tc.tile_pool(name="pp", bufs=4))
            ob_pool = att.enter_context(tc.tile_pool(name="ob", bufs=4))
            o_pool = att.enter_context(tc.tile_pool(name="op", bufs=4))
            bc_pool = att.enter_context(tc.tile_pool(name="bc", bufs=4))
            sm_pool = att.enter_context(tc.tile_pool(name="sm", bufs=4))
            # PSUM: 2x scores staging (2 banks each) + 2x av + 2x shared
            # companion/bps slots = 8 banks
            psum = att.enter_context(tc.tile_pool(name="psum", bufs=2,
                                                  space="PSUM"))
            avps = att.enter_context(tc.tile_pool(name="avps", bufs=2,
                                                  space="PSUM"))
            cps = att.enter_context(tc.tile_pool(name="cps", bufs=2,
                                                 space="PSUM"))

            def load_wqk(wqk_d, rnd):
                wt = []
                for kc in range(KC):
                    t_ = wqk_pool.tile([128, 2 * CP], BF16,
                                       name=f"wqk{rnd}_{kc}", tag="wqk")
                    eng = nc.scalar if kc % 2 else nc.sync
                    eng.dma_start(t_[:], wqk_d[128 * kc:128 * (kc + 1), :])
                    wt.append(t_)
                return wt

            def load_wv(wv_d, rnd):
                wt = []
                for kc in range(KC):
                    t_ = wv_pool.tile([128, CP], BF16,
                                      name=f"wv{rnd}_{kc}", tag="wv")
                    eng = nc.scalar if kc % 2 else nc.sync
                    eng.dma_start(t_[:], wv_d[128 * kc:128 * (kc + 1), :])
                    wt.append(t_)
                return wt

            def qk_tiles(rnd):
                return [qk_pool.tile([128, T], BF16, name=f"qk{rnd}_{jc}",
                                     tag="qk") for jc in range(4)]

            def proj_qk_unit(wt, src, biasc, qk, jc, t4):
                """One [128,512] q^T/k^T block: 8 matmuls + bias add."""
                ps = cps.tile([128, 512], F32, tag="cps", name="ps_pqk")
                for kc in range(KC):
                    nc.tensor.matmul(
                        ps[:], wt[kc][:, 128 * jc:128 * (jc + 1)],
                        src[kc][:, 512 * t4:512 * (t4 + 1)],
                        start=(kc == 0), stop=(kc == KC - 1))
                nc.vector.tensor_scalar_add(
                    qk[jc][:, 512 * t4:512 * (t4 + 1)], ps[:],
                    biasc[:, jc:jc + 1])

            def proj_v_unit(wt, src, va, rnd, tb):
                """One 128-token block of v (natural layout + ones col)."""
                t_ = vaug_pool.tile([128, HG, DH + 1], BF16,
                                    name=f"va{rnd}_{tb}", tag="va")
                ps = cps.tile([128, 512], F32, tag="cps", name="ps_pv")
                for kc in range(KC):
                    nc.tensor.matmul(
                        ps[:, 0:CP], src[kc][:, 128 * tb:128 * (tb + 1)],
                        wt[kc][:], start=(kc == 0), stop=(kc == KC - 1))
                nc.vector.tensor_copy(
                    t_[:, :, 0:DH],
                    ps[:, 0:CP].rearrange("p (h d) -> p h d", h=HG))
                nc.vector.tensor_copy(t_[:, :, DH:DH + 1],
                                      ones_sb[:, :, None])
                va.append(t_)

            def attend_iter(qk, va, biasv, ob, q4, hh):
                """One (q4, hh) attention iteration, software-pipelined:
                scores+exp run one key block ahead of the AV matmuls."""
                nblk = 4 * (q4 + 1)
                qt, kt = qk[hh], qk[2 + hh]
                av = [avps.tile([DH + 1, 512], F32, tag="av",
                                name=f"av{h2}") for h2 in range(2)]
                pending = []

                def emit_scores(kb):
                    diag = kb - 4 * q4
                    c0 = max(0, 128 * diag)
                    npr = 512 - c0
                    ps = psum.tile([128, 2, 512], F32, tag="ps", name="s2")
                    for h2 in range(2):
                        base = 64 * h2
                        nc.tensor.matmul(
                            ps[:, h2, 0:npr],
                            kt[base:base + 64, 128 * kb:128 * (kb + 1)],
                            qt[base:base + 64,
                               512 * q4 + c0:512 * (q4 + 1)],
                            start=True, stop=True)
                    p = p_pool.tile([128, 2, 512], BF16, tag="p", name="p2")
                    nc.scalar.activation(p[:, :, 0:npr], ps[:, :, 0:npr],
                                         EXP, scale=SCALE)
                    if diag >= 0:
                        for h2 in range(2):
                            nc.vector.tensor_mul(
                                p[:, h2, 0:128], p[:, h2, 0:128], mask_sb[:])
                    pending.append((p, kb, c0, npr))

                def emit_av():
                    p, kb, c0, npr = pending.pop(0)
                    for h2 in range(2):
                        h = 2 * hh + h2
                        nc.tensor.matmul(
                            av[h2][:, c0:512], va[kb][:, h, :],
                            p[:, h2, 0:npr],
                            start=(kb == 0), stop=(kb == nblk - 1))

                for kb in range(nblk):
                    emit_scores(kb)
                    if len(pending) == 2:
                        emit_av()
                while pending:
                    emit_av()

                recr = sm_pool.tile([1, 1024], BF16, tag="recr", name="recr")
                with nc.allow_low_precision(
                        reason="softmax recip rounds to bf16"):
                    nc.vector.reciprocal(recr[0:1, 0:512],
                                         av[0][DH:DH + 1, :])
                    nc.vector.reciprocal(recr[0:1, 512:1024],
                                         av[1][DH:DH + 1, :])
                bps = cps.tile([128, 512], F32, tag="cps", name="bps")
                for h2 in range(2):
                    nc.tensor.matmul(
                        bps[64 * h2:64 * (h2 + 1), :], ones64_sb[:],
                        recr[0:1, 512 * h2:512 * (h2 + 1)],
                        start=True, stop=True)
                bc = bc_pool.tile([128, 512], F32, tag="bc", name="bc")
                nc.vector.tensor_copy(bc[:], bps[:])
                osl = ob[hh][:, 512 * q4:512 * (q4 + 1)]
                for h2 in range(2):
                    nc.vector.tensor_mul(
                        osl[64 * h2:64 * (h2 + 1), :],
                        av[h2][0:DH, :],
                        bc[64 * h2:64 * (h2 + 1), :])
                nc.vector.tensor_scalar_add(osl, osl, biasv[:, hh:hh + 1])

            def run_iters(iters, comps, after=None):
                """Emit attention iterations with companion units spread
                between them (fills the PE while the Act engine works)."""
                comps = list(comps)
                n = len(iters)
                for i, (fn, args) in enumerate(iters):
                    fn(*args)
                    left = n - i - 1
                    if comps:
                        take = (len(comps) if left == 0
                                else max(1, len(comps) // (left + 1)))
                        for _ in range(take):
                            if comps:
                                comps.pop(0)()
                    if after is not None:
                        after(i)
                for c in comps:
                    c()

            def emit_pred(half, ob):
                cs = slice(TH * half, TH * (half + 1))
                for hh in range(2):
                    nc.sync.dma_start(
                        pred_part[half][128 * hh:128 * (hh + 1), :],
                        ob[hh][:, cs])
                if sim:
                    nc.sync.dma_start(pred_full[half, 0:CP, :],
                                      pred_part[half][:, :])
                else:
                    nc.gpsimd.collective_compute(
                        "AllGather", mybir.AluOpType.bypass,
                        replica_groups=GROUPS,
                        ins=[pred_part[half][:, :]],
                        outs=[pred_full[half, :, :]])

            def emit_corr(hh, obh):
                for s in range(G):
                    for dup in range(2):
                        c_ = 4 * dup + s
                        nc.sync.dma_start(
                            corr_a2a[hh][128 * c_:128 * (c_ + 1), :],
                            obh[:, 512 * s:512 * (s + 1)])
                if sim:
                    nc.sync.dma_start(corr_out[hh][0:128, 0:64],
                                      corr_a2a[hh][0:128, 0:64])
                else:
                    nc.gpsimd.collective_compute(
                        "AllToAll", mybir.AluOpType.bypass,
                        replica_groups=GROUP8,
                        ins=[corr_a2a[hh][:, :]],
                        outs=[corr_out[hh][:, :]])

            def resid_unit(hf, kc):
                def f():
                    cs = slice(TH * hf, TH * (hf + 1))
                    pt = o_pool.tile([128, TH], BF16, tag="op", name="predld")
                    nc.sync.dma_start(
                        pt[:], pred_full[hf, 128 * kc:128 * (kc + 1), :])
                    nc.vector.tensor_sub(xt[kc][:, cs], xt[kc][:, cs], pt[:])
                return f

            # ================= phase A: round-0 projection ==============
            wv0_t = load_wv(wv0, 0)
            for hf in range(4):
                for kc in range(KC):
                    eng = nc.scalar if (kc + hf) % 2 else nc.sync
                    eng.dma_start(xt[kc][:, 512 * hf:512 * (hf + 1)],
                                  xT[128 * kc:128 * (kc + 1),
                                     512 * hf:512 * (hf + 1)])
                if hf == 0:
                    wqk0_t = load_wqk(wqk0, 0)
            va0 = []
            qk0 = qk_tiles(0)
            # emit only what attn0's first token half needs, the rest
            # becomes companion work under attention
            for tb in range(8):
                proj_v_unit(wv0_t, xt, va0, 0, tb)
            for t4 in range(2):
                for jc in range(4):
                    proj_qk_unit(wqk0_t, xt, bqk_sb[0], qk0, jc, t4)

            ob0 = [ob_pool.tile([128, T], BF16, tag="ob", name=f"ob0_{hh}")
                   for hh in range(2)]
            ob1 = [ob_pool.tile([128, T], BF16, tag="ob", name=f"ob1_{hh}")
                   for hh in range(2)]

            # ============ phase B: attn0 first token half ===============
            it0 = lambda q4, hh: (attend_iter, (qk0, va0, bv_sb[0], ob0,
                                                q4, hh))
            comps = [(lambda tb=tb: proj_v_unit(wv0_t, xt, va0, 0, tb))
                     for tb in range(8, 16)]
            for t4 in range(2, 4):
                comps += [(lambda jc=jc, t4=t4: proj_qk_unit(
                    wqk0_t, xt, bqk_sb[0], qk0, jc, t4)) for jc in range(4)]
            run_iters([it0(0, 0), it0(0, 1), it0(1, 0), it0(1, 1)], comps)
            emit_pred(0, ob0)

            # ============ phase C: attn0 second half + round-1 proj =====
            wv1_t = load_wv(wv1, 1)
            wqk1_t = load_wqk(wqk1, 1)
            va1 = []
            qk1 = qk_tiles(1)
            comps = [resid_unit(0, kc) for kc in range(KC)]
            comps += [(lambda tb=tb: proj_v_unit(wv1_t, xt, va1, 1, tb))
                      for tb in range(8)]
            for t4 in range(2):
                comps += [(lambda jc=jc, t4=t4: proj_qk_unit(
                    wqk1_t, xt, bqk_sb[1], qk1, jc, t4)) for jc in range(4)]
            run_iters([it0(2, 0), it0(2, 1), it0(3, 0), it0(3, 1)], comps)
            emit_pred(1, ob0)

            # ============ phase D: attn1 first half + rest of proj ======
            # prefetch gate-phase pred strips (local read of the AG output)
            pred_v = pred_full[:, :, :].rearrange("h d (s t) -> h d s t",
                                                  s=2)
            predg = []
            for cc in range(KC):
                pg_ = wqk_pool.tile([128, TS], BF16, name=f"predg{cc}",
                                    tag="wqk")
                nc.gpsimd.dma_start(
                    pg_[:], pred_v[bass.ds(hf_sel, 1),
                                   128 * cc:128 * (cc + 1),
                                   bass.ds(st_sel, 1), :]
                    .squeeze(2).squeeze(0))
                predg.append(pg_)

            wg_t = []
            for i in range(KC):
                t_ = big8.tile([128, 2 * D], BF16, name=f"wg{i}", tag="b8")
                nc.gpsimd.dma_start(
                    t_[:].rearrange("p (a d) -> p a d", a=2),
                    bass.AP(tensor=wg, offset=256 * i * D,
                            ap=[[D, 128], [128 * D, 2], [1, D]]))
                wg_t.append(t_)
            wo_t = []
            for cc in range(KC):
                t_ = wo_pool.tile([128, D], BF16, name=f"wo{cc}", tag="wo")
                nc.sync.dma_start(t_[:], wo[128 * cc:128 * (cc + 1), :])
                wo_t.append(t_)

            it1 = lambda q4, hh: (attend_iter, (qk1, va1, bv_sb[1], ob1,
                                                q4, hh))
            comps = [resid_unit(1, kc) for kc in range(KC)]
            comps += [(lambda tb=tb: proj_v_unit(wv1_t, xt, va1, 1, tb))
                      for tb in range(8, 16)]
            for t4 in range(2, 4):
                comps += [(lambda jc=jc, t4=t4: proj_qk_unit(
                    wqk1_t, xt, bqk_sb[1], qk1, jc, t4)) for jc in range(4)]
            run_iters([it1(0, 0), it1(1, 0), it1(0, 1), it1(1, 1)], comps)

            # ============ phase E: attn1 second half + gate pred part ===
            gp_pre = [None] * KC

            def wg_slice(cc, jc):
                return wg_t[cc // 2][:, D * (cc % 2) + 128 * jc:
                                     D * (cc % 2) + 128 * (jc + 1)]

            def inject_gate(jc):
                def f():
                    ps = cps.tile([128, 512], F32, tag="cps", name="gp_ps")
                    for cc in range(KC):
                        nc.tensor.matmul(ps[:], wg_slice(cc, jc),
                                         predg[cc][:],
                                         start=(cc == 0), stop=(cc == KC - 1))
                    t_ = gpre_pool.tile([128, TS], F32, name=f"gpre{jc}",
                                        tag="gpre")
                    nc.vector.tensor_copy(t_[:], ps[:])
                    gp_pre[jc] = t_
                return f

            comps = [inject_gate(jc) for jc in range(KC)]

            def after_e(i):
                if i == 1:
                    emit_corr(0, ob1[0])
                elif i == 3:
                    emit_corr(1, ob1[1])

            run_iters([it1(2, 0), it1(3, 0), it1(2, 1), it1(3, 1)], comps,
                      after=after_e)

        # ================= phase F: gate + output ==================
        with ExitStack() as gat:
            gp_pool = gat.enter_context(tc.tile_pool(name="gp", bufs=8))
            y_pool = gat.enter_context(tc.tile_pool(name="yp", bufs=2))
            ps2 = gat.enter_context(tc.tile_pool(name="ps2", bufs=8,
                                                 space="PSUM"))

            # corr strip: channel 128cc belongs to group member cc//2,
            # head pair cc%2 — even chunks land with A2A half 0, odd with
            # half 1; consume in that order so the tail A2A overlaps the
            # gate matmul
            corr_bv = [t[:, :].rearrange("(b r) t -> b r t", b=2)
                       for t in corr_out]
            corr_t = [qk_pool.tile([128, T], BF16, name=f"corrt{i}", tag="qk")
                      for i in range(2)]
            for cc in [0, 2, 4, 6, 1, 3, 5, 7]:
                r0 = 128 * (cc // 2)
                nc.sync.dma_start(
                    corr_t[cc // 4][:, 512 * (cc % 4):512 * (cc % 4 + 1)],
                    corr_bv[cc % 2][bass.ds(bsel_y, 1),
                                    r0:r0 + 128, :].squeeze(0))
            corrg = [corr_t[cc // 4][:, 512 * (cc % 4):512 * (cc % 4 + 1)]
                     for cc in range(KC)]

            pgt = []
            for jc in range(KC):
                ps = ps2.tile([128, 512], F32, tag="ps2", name=f"ps_g{jc}")
                for i, cc in enumerate([0, 2, 4, 6, 1, 3, 5, 7]):
                    nc.tensor.matmul(ps[:], wg_slice(KC + cc, jc), corrg[cc],
                                     start=(i == 0), stop=(i == KC - 1))
                # fold in the pred-half partial computed during attention 1
                nc.vector.tensor_add(ps[:], ps[:], gp_pre[jc][:])
                gt = gp_pool.tile([128, TS], BF16, name=f"gate{jc}", tag="gp")
                nc.scalar.activation(gt[:], ps[:], SIG,
                                     bias=bg_sb[:, jc:jc + 1])
                nc.vector.tensor_mul(gt[:], gt[:], corrg[jc])
                nc.vector.tensor_add(gt[:], gt[:], predg[jc][:])
                pgt.append(gt)

            for tb in range(4):
                yt = y_pool.tile([128, D], F32, tag="y", name="yt")
                for n2 in range(2):
                    ps = ps2.tile([128, 512], F32, tag="ps2", name="ps_y")
                    for cc in range(KC):
                        nc.tensor.matmul(
                            ps[:], pgt[cc][:, 128 * tb:128 * (tb + 1)],
                            wo_t[cc][:, 512 * n2:512 * (n2 + 1)],
                            start=(cc == 0), stop=(cc == KC - 1))
                    nc.vector.tensor_copy(yt[:, 512 * n2:512 * (n2 + 1)],
                                          ps[:])
                nc.sync.dma_start(y[128 * tb:128 * (tb + 1), :], yt[:])


_NC = None


def _get_nc():
    global _NC
    if _NC is None:
        _NC = _build()
    return _NC


def make_in_maps(x, Wqkv0, bqkv0, Wqkv1, bqkv1, Wg, bg, Wo, bo):
    bf = mybir.dt.np(BF16)
    mask_np = np.where(np.arange(128)[:, None] > np.arange(128)[None, :],
                       0.0, 1.0).astype(bf)
    ones_np = np.ones((128, HG), bf)
    ones64_np = np.ones((1, 64), bf)
    bg_a = np.ascontiguousarray(bg.reshape(D // 128, 128).T.astype(np.float32))
    wg_np = np.ascontiguousarray(Wg.astype(np.float32).astype(bf))
    wo_np = np.ascontiguousarray(Wo.astype(np.float32).astype(bf))

    in_maps = []
    for c in range(8):
        b, g = divmod(c, G)
        cq = slice(CP * g, CP * (g + 1))
        ck = slice(D + CP * g, D + CP * (g + 1))
        cv = slice(2 * D + CP * g, 2 * D + CP * (g + 1))
        m = {
            "xT": np.ascontiguousarray(x[b].T.astype(np.float32).astype(bf)),
            "mask01": mask_np, "onesc": ones_np, "bg": bg_a,
            "ones64": ones64_np,
            "wg": wg_np, "wo": wo_np,
        }
        for r, (W, bb) in enumerate(((Wqkv0, bqkv0), (Wqkv1, bqkv1))):
            m[f"wqk{r}"] = np.ascontiguousarray(
                np.concatenate([W[:, cq], W[:, ck]], axis=1)
                .astype(np.float32).astype(bf))
            m[f"wv{r}"] = np.ascontiguousarray(
                W[:, cv].astype(np.float32).astype(bf))
            bqk_cat = np.concatenate([bb[cq], bb[ck]]).astype(np.float32)
            m[f"bqk{r}"] = np.ascontiguousarray(bqk_cat.reshape(4, 128).T)
            m[f"bv{r}"] = np.ascontiguousarray(
                bb[cv].astype(np.float32).reshape(2, 128).T)
        in_maps.append(m)
    return in_maps


def assemble(results, bo):
    out = np.empty((B, T, D), np.float32)
    for c in range(8):
        b, g = divmod(c, G)
        out[b, TS * g:TS * (g + 1), :] = results[c]["y"]
    return out + bo.astype(np.float32)


def kernel(x, Wqkv0, bqkv0, Wqkv1, bqkv1, Wg, bg, Wo, bo):
    args = [np.asarray(a) for a in
            (x, Wqkv0, bqkv0, Wqkv1, bqkv1, Wg, bg, Wo, bo)]
    nc = _get_nc()
    in_maps = make_in_maps(*args)
    res = bass_utils.run_bass_kernel_spmd(nc, in_maps, core_ids=list(range(8)))
    return assemble(res.results, args[8])


# revision 3
# speedup vs baseline: 1.0870x; 1.0267x over previous
"""BoostedCausalAttention on 8 trn2 NeuronCores — software-pipelined bf16.

Sharding: core c -> (batch b = c//4, head-group g = c%4, 4 heads each).
Within a 4-core batch group (Megatron-style):
  - qkv projections + attention in "transposed" layout (feature on
    partitions, token on free axis), bf16 matmuls with f32 PSUM accum.
  - The attention inner loop is software-pipelined: the scores matmul for
    key block kb+1 is emitted BEFORE the AV matmul of block kb, so the PE
    never head-of-line blocks on the exp; exp covers both heads of a pair
    in one [128,2,npr] activation.
  - Cross-phase pipelining: round-1 projection interleaves into round-0
    attention's tail; the gate's pred-half matmuls interleave into
    round-1 attention; AllGathers/AllToAlls fire per token-half /
    head-pair so their latency hides under compute.
  - Final output: each core emits y[512 tokens, 1024] f32 (pre-bias);
    host concatenates and adds bo.
"""

from contextlib import ExitStack

import numpy as np

import concourse.bass as bass
import concourse.bacc as bacc
import concourse.tile as tile
import concourse.mybir as mybir
from concourse import bass_utils

B, T, D = 2, 2048, 1024
H, DH = 16, 64
SCALE = DH ** -0.5
G = 4            # head groups (cores per batch)
HG = H // G      # heads per core = 4
CP = HG * DH     # channels per core = 256
TS = T // G      # token slice per core for gate/output phase = 512
KC = D // 128    # contraction chunks over D = 8
TH = T // 2      # token half = 1024

F32 = mybir.dt.float32
BF16 = mybir.dt.bfloat16
EXP = mybir.ActivationFunctionType.Exp
SIG = mybir.ActivationFunctionType.Sigmoid

GROUPS = [[0, 1, 2, 3], [4, 5, 6, 7]]
GROUP8 = [[0, 1, 2, 3, 4, 5, 6, 7]]


def _build(sim=False, reps=1):
    nc = bacc.Bacc("TRN2", target_bir_lowering=False, debug=False, num_devices=8)

    xT = nc.dram_tensor("xT", [D, T], BF16, kind="ExternalInput")
    wqk0 = nc.dram_tensor("wqk0", [D, 2 * CP], BF16, kind="ExternalInput")
    wv0 = nc.dram_tensor("wv0", [D, CP], BF16, kind="ExternalInput")
    bqk0 = nc.dram_tensor("bqk0", [128, 4], F32, kind="ExternalInput")
    bv0 = nc.dram_tensor("bv0", [128, 2], F32, kind="ExternalInput")
    wqk1 = nc.dram_tensor("wqk1", [D, 2 * CP], BF16, kind="ExternalInput")
    wv1 = nc.dram_tensor("wv1", [D, CP], BF16, kind="ExternalInput")
    bqk1 = nc.dram_tensor("bqk1", [128, 4], F32, kind="ExternalInput")
    bv1 = nc.dram_tensor("bv1", [128, 2], F32, kind="ExternalInput")
    wg = nc.dram_tensor("wg", [2 * D, D], BF16, kind="ExternalInput")
    bg = nc.dram_tensor("bg", [128, D // 128], F32, kind="ExternalInput")
    wo = nc.dram_tensor("wo", [D, D], BF16, kind="ExternalInput")
    mask01 = nc.dram_tensor("mask01", [128, 128], BF16, kind="ExternalInput")
    onesc = nc.dram_tensor("onesc", [128, HG], BF16, kind="ExternalInput")
    ones64 = nc.dram_tensor("ones64", [1, 64], BF16, kind="ExternalInput")
    y = nc.dram_tensor("y", [TS, D], F32, kind="ExternalOutput")

    with tile.TileContext(nc) as tc, ExitStack() as glb:
        consts = glb.enter_context(tc.tile_pool(name="consts", bufs=1))
        # 4KB-per-partition slots: x^T/residual tiles, later reused by Wg
        big8 = glb.enter_context(tc.tile_pool(name="big8", bufs=8))
        dpool = glb.enter_context(tc.tile_pool(name="dpool", bufs=1, space="DRAM"))

        # ---- constants ----
        bqk_sb = [consts.tile([128, 4], F32, name=f"bqk_sb{r}") for r in range(2)]
        bv_sb = [consts.tile([128, 2], F32, name=f"bv_sb{r}") for r in range(2)]
        bg_sb = consts.tile([128, D // 128], F32)
        mask_sb = consts.tile([128, 128], BF16)
        ones_sb = consts.tile([128, HG], BF16)
        ones64_sb = consts.tile([1, 64], BF16)
        nc.gpsimd.dma_start(ones_sb[:], onesc[:, :])
        nc.gpsimd.dma_start(ones64_sb[:], ones64[:, :])
        nc.gpsimd.dma_start(bqk_sb[0][:], bqk0[:, :])
        nc.gpsimd.dma_start(bqk_sb[1][:], bqk1[:, :])
        nc.gpsimd.dma_start(bv_sb[0][:], bv0[:, :])
        nc.gpsimd.dma_start(bv_sb[1][:], bv1[:, :])
        nc.gpsimd.dma_start(bg_sb[:], bg[:, :])
        nc.gpsimd.dma_start(mask_sb[:], mask01[:, :])

        # ---- collective DRAM buffers ----
        pred_part = [dpool.tile([CP, TH], BF16, name=f"pred_part{i}")
                     for i in range(2)]
        pred_full = dpool.tile([2, D, TH], BF16, name="pred_full")
        corr_a2a = [dpool.tile([8 * 128, TS], BF16, name=f"corr_a2a{i}")
                    for i in range(2)]
        corr_out = [dpool.tile([8 * 128, TS], BF16, name=f"corr_out{i}")
                    for i in range(2)]

        wqk_pool = glb.enter_context(tc.tile_pool(name="wqk", bufs=2 * KC))
        qk_pool = glb.enter_context(tc.tile_pool(name="qk", bufs=9))
        wo_pool = glb.enter_context(tc.tile_pool(name="wo", bufs=1))
        wg_pool = glb.enter_context(tc.tile_pool(name="wg", bufs=1))
        gpre_pool = glb.enter_context(tc.tile_pool(name="gpre", bufs=KC))

        # gate/output weights are SBUF-resident for the kernel's lifetime:
        # loaded once, so the xt pool slots never serialize through them
        wg_t = []
        for i in range(KC):
            t_ = wg_pool.tile([128, 2 * D], BF16, name=f"wg{i}")
            nc.gpsimd.dma_start(
                t_[:].rearrange("p (a d) -> p a d", a=2),
                bass.AP(tensor=wg, offset=256 * i * D,
                        ap=[[D, 128], [128 * D, 2], [1, D]]))
            wg_t.append(t_)
        wo_t = []
        for cc in range(KC):
            t_ = wo_pool.tile([128, D], BF16, name=f"wo{cc}")
            eng = nc.scalar if cc % 2 else nc.sync
            eng.dma_start(t_[:], wo[128 * cc:128 * (cc + 1), :])
            wo_t.append(t_)

        # strip-select registers (snap donates in place); set up once.
        # g = pid%4: half hf = g//2, within-half slot st = g%2.
        pid = nc.gpsimd.partition_id()
        r_t = nc.gpsimd.alloc_register("selt")
        nc.gpsimd.reg_div(r_t, pid, 2)
        half_sel = nc.gpsimd.snap(r_t, donate=True, min_val=0, max_val=3)
        r_hf = nc.gpsimd.alloc_register("selhf")
        nc.gpsimd.reg_mod(r_hf, half_sel, 2)
        hf_sel = nc.gpsimd.snap(r_hf, donate=True, min_val=0, max_val=1)
        r_st = nc.gpsimd.alloc_register("selst")
        nc.gpsimd.reg_mod(r_st, pid, 2)
        st_sel = nc.gpsimd.snap(r_st, donate=True, min_val=0, max_val=1)

        pid_y = nc.sync.partition_id()
        r_by = nc.sync.alloc_register("selby")
        nc.sync.reg_div(r_by, pid_y, G)
        bsel_y = nc.sync.snap(r_by, donate=True, min_val=0, max_val=1)

        _body(nc, tc, sim, reps, big8, wqk_pool, qk_pool, wg_t, wo_t, gpre_pool,
              bqk_sb, bv_sb, bg_sb, mask_sb, ones_sb, ones64_sb,
              hf_sel, st_sel, bsel_y, xT, wqk0, wv0, wqk1, wv1, y,
              pred_part, pred_full, corr_a2a, corr_out)

    nc.compile()
    return nc


def _body(nc, tc, sim, reps, big8, wqk_pool, qk_pool, wg_t, wo_t, gpre_pool,
          bqk_sb, bv_sb, bg_sb, mask_sb, ones_sb, ones64_sb,
          hf_sel, st_sel, bsel_y, xT, wqk0, wv0, wqk1, wv1, y,
          pred_part, pred_full, corr_a2a, corr_out):
    pending_tail = None
    for _rep in range(reps):
        xt = []
        for kc in range(KC):
            xt.append(big8.tile([128, T], BF16, name=f"xt{kc}", tag="b8"))
        with ExitStack() as att:
            wv_pool = att.enter_context(tc.tile_pool(name="wv", bufs=KC))
            vaug_pool = att.enter_context(tc.tile_pool(name="vaug", bufs=24))
            p_pool = att.enter_context(tc.tile_pool(name="pp", bufs=4))
            ob_pool = att.enter_context(tc.tile_pool(name="ob", bufs=4))
            o_pool = att.enter_context(tc.tile_pool(name="op", bufs=3))
            bc_pool = att.enter_context(tc.tile_pool(name="bc", bufs=2))
            sm_pool = att.enter_context(tc.tile_pool(name="sm", bufs=2))
            # PSUM: 2x scores staging (2 banks each) + 2x av + 2x shared
            # companion/bps slots = 8 banks
            psum = att.enter_context(tc.tile_pool(name="psum", bufs=2,
                                                  space="PSUM"))
            avps = att.enter_context(tc.tile_pool(name="avps", bufs=2,
                                                  space="PSUM"))
            cps = att.enter_context(tc.tile_pool(name="cps", bufs=2,
                                                 space="PSUM"))

            def load_wqk(wqk_d, rnd):
                wt = []
                for kc in range(KC):
                    t_ = wqk_pool.tile([128, 2 * CP], BF16,
                                       name=f"wqk{rnd}_{kc}", tag="wqk")
                    eng = nc.scalar if kc % 2 else nc.sync
                    eng.dma_start(t_[:], wqk_d[128 * kc:128 * (kc + 1), :])
                    wt.append(t_)
                return wt

            def load_wv(wv_d, rnd):
                wt = []
                for kc in range(KC):
                    t_ = wv_pool.tile([128, CP], BF16,
                                      name=f"wv{rnd}_{kc}", tag="wv")
                    eng = nc.scalar if kc % 2 else nc.sync
                    eng.dma_start(t_[:], wv_d[128 * kc:128 * (kc + 1), :])
                    wt.append(t_)
                return wt

            def qk_tiles(rnd):
                return [qk_pool.tile([128, T], BF16, name=f"qk{rnd}_{jc}",
                                     tag="qk") for jc in range(4)]

            def proj_qk_unit(wt, src, biasc, qk, jc, t4):
                """One [128,512] q^T/k^T block: 8 matmuls + bias add."""
                ps = cps.tile([128, 512], F32, tag="cps", name="ps_pqk")
                for kc in range(KC):
                    nc.tensor.matmul(
                        ps[:], wt[kc][:, 128 * jc:128 * (jc + 1)],
                        src[kc][:, 512 * t4:512 * (t4 + 1)],
                        start=(kc == 0), stop=(kc == KC - 1))
                nc.vector.tensor_scalar_add(
                    qk[jc][:, 512 * t4:512 * (t4 + 1)], ps[:],
                    biasc[:, jc:jc + 1])

            def proj_v_unit(wt, src, va, rnd, tb):
                """One 128-token block of v (natural layout + ones col)."""
                t_ = vaug_pool.tile([128, HG, DH + 1], BF16,
                                    name=f"va{rnd}_{tb}", tag="va")
                ps = cps.tile([128, 512], F32, tag="cps", name="ps_pv")
                for kc in range(KC):
                    nc.tensor.matmul(
                        ps[:, 0:CP], src[kc][:, 128 * tb:128 * (tb + 1)],
                        wt[kc][:], start=(kc == 0), stop=(kc == KC - 1))
                nc.vector.tensor_copy(
                    t_[:, :, 0:DH],
                    ps[:, 0:CP].rearrange("p (h d) -> p h d", h=HG))
                nc.vector.tensor_copy(t_[:, :, DH:DH + 1],
                                      ones_sb[:, :, None])
                va.append(t_)

            def attend_iter(qk, va, biasv, ob, q4, hh):
                """One (q4, hh) attention iteration, software-pipelined:
                scores+exp run one key block ahead of the AV matmuls."""
                nblk = 4 * (q4 + 1)
                qt, kt = qk[hh], qk[2 + hh]
                av = [avps.tile([DH + 1, 512], F32, tag="av",
                                name=f"av{h2}") for h2 in range(2)]
                pending = []

                def emit_scores(kb):
                    diag = kb - 4 * q4
                    c0 = max(0, 128 * diag)
                    npr = 512 - c0
                    ps = psum.tile([128, 2, 512], F32, tag="ps", name="s2")
                    for h2 in range(2):
                        base = 64 * h2
                        nc.tensor.matmul(
                            ps[:, h2, 0:npr],
                            kt[base:base + 64, 128 * kb:128 * (kb + 1)],
                            qt[base:base + 64,
                               512 * q4 + c0:512 * (q4 + 1)],
                            start=True, stop=True)
                    p = p_pool.tile([128, 2, 512], BF16, tag="p", name="p2")
                    nc.scalar.activation(p[:, :, 0:npr], ps[:, :, 0:npr],
                                         EXP, scale=SCALE)
                    if diag >= 0:
                        for h2 in range(2):
                            nc.vector.tensor_mul(
                                p[:, h2, 0:128], p[:, h2, 0:128], mask_sb[:])
                    pending.append((p, kb, c0, npr))

                def emit_av():
                    p, kb, c0, npr = pending.pop(0)
                    for h2 in range(2):
                        h = 2 * hh + h2
                        nc.tensor.matmul(
                            av[h2][:, c0:512], va[kb][:, h, :],
                            p[:, h2, 0:npr],
                            start=(kb == 0), stop=(kb == nblk - 1))

                for kb in range(nblk):
                    emit_scores(kb)
                    if len(pending) == 2:
                        emit_av()
                while pending:
                    emit_av()

                recr = sm_pool.tile([1, 1024], BF16, tag="recr", name="recr")
                with nc.allow_low_precision(
                        reason="softmax recip rounds to bf16"):
                    nc.vector.reciprocal(recr[0:1, 0:512],
                                         av[0][DH:DH + 1, :])
                    nc.vector.reciprocal(recr[0:1, 512:1024],
                                         av[1][DH:DH + 1, :])
                bps = cps.tile([128, 512], F32, tag="cps", name="bps")
                for h2 in range(2):
                    nc.tensor.matmul(
                        bps[64 * h2:64 * (h2 + 1), :], ones64_sb[:],
                        recr[0:1, 512 * h2:512 * (h2 + 1)],
                        start=True, stop=True)
                bc = bc_pool.tile([128, 512], F32, tag="bc", name="bc")
                nc.vector.tensor_copy(bc[:], bps[:])
                osl = ob[hh][:, 512 * q4:512 * (q4 + 1)]
                for h2 in range(2):
                    nc.vector.tensor_mul(
                        osl[64 * h2:64 * (h2 + 1), :],
                        av[h2][0:DH, :],
                        bc[64 * h2:64 * (h2 + 1), :])
                nc.vector.tensor_scalar_add(osl, osl, biasv[:, hh:hh + 1])

            def run_iters(iters, comps, after=None):
                """Emit attention iterations with companion units spread
                between them (fills the PE while the Act engine works)."""
                comps = list(comps)
                n = len(iters)
                for i, (fn, args) in enumerate(iters):
                    fn(*args)
                    left = n - i - 1
                    if comps:
                        take = (len(comps) if left == 0
                                else max(1, len(comps) // (left + 1)))
                        for _ in range(take):
                            if comps:
                                comps.pop(0)()
                    if after is not None:
                        after(i)
                for c in comps:
                    c()

            def emit_pred(half, ob):
                cs = slice(TH * half, TH * (half + 1))
                for hh in range(2):
                    nc.sync.dma_start(
                        pred_part[half][128 * hh:128 * (hh + 1), :],
                        ob[hh][:, cs])
                if sim:
                    nc.sync.dma_start(pred_full[half, 0:CP, :],
                                      pred_part[half][:, :])
                else:
                    nc.gpsimd.collective_compute(
                        "AllGather", mybir.AluOpType.bypass,
                        replica_groups=GROUPS,
                        ins=[pred_part[half][:, :]],
                        outs=[pred_full[half, :, :]])

            def emit_corr(hh, obh):
                for s in range(G):
                    for dup in range(2):
                        c_ = 4 * dup + s
                        nc.sync.dma_start(
                            corr_a2a[hh][128 * c_:128 * (c_ + 1), :],
                            obh[:, 512 * s:512 * (s + 1)])
                if sim:
                    nc.sync.dma_start(corr_out[hh][0:128, 0:64],
                                      corr_a2a[hh][0:128, 0:64])
                else:
                    nc.gpsimd.collective_compute(
                        "AllToAll", mybir.AluOpType.bypass,
                        replica_groups=GROUP8,
                        ins=[corr_a2a[hh][:, :]],
                        outs=[corr_out[hh][:, :]])

            def resid_unit(hf, kc):
                def f():
                    cs = slice(TH * hf, TH * (hf + 1))
                    pt = o_pool.tile([128, TH], BF16, tag="op", name="predld")
                    nc.sync.dma_start(
                        pt[:], pred_full[hf, 128 * kc:128 * (kc + 1), :])
                    nc.vector.tensor_sub(xt[kc][:, cs], xt[kc][:, cs], pt[:])
                return f

            # ================= phase A: round-0 projection ==============
            wv0_t = load_wv(wv0, 0)
            for hf in range(4):
                for kc in range(KC):
                    eng = nc.scalar if (kc + hf) % 2 else nc.sync
                    eng.dma_start(xt[kc][:, 512 * hf:512 * (hf + 1)],
                                  xT[128 * kc:128 * (kc + 1),
                                     512 * hf:512 * (hf + 1)])
                if hf == 0:
                    wqk0_t = load_wqk(wqk0, 0)
            va0 = []
            qk0 = qk_tiles(0)
            # emit only what attn0's first token half needs, the rest
            # becomes companion work under attention
            for tb in range(8):
                proj_v_unit(wv0_t, xt, va0, 0, tb)
            for t4 in range(2):
                for jc in range(4):
                    proj_qk_unit(wqk0_t, xt, bqk_sb[0], qk0, jc, t4)

            # previous rep's gate/output tail: emitted here so its corr
            # A2A latency and PE tail overlap this rep's projection start
            if pending_tail is not None:
                pending_tail(cps)
                pending_tail = None

            ob0 = [ob_pool.tile([128, T], BF16, tag="ob", name=f"ob0_{hh}")
                   for hh in range(2)]
            ob1 = [ob_pool.tile([128, T], BF16, tag="ob", name=f"ob1_{hh}")
                   for hh in range(2)]

            # ============ phase B: attn0 first token half ===============
            it0 = lambda q4, hh: (attend_iter, (qk0, va0, bv_sb[0], ob0,
                                                q4, hh))
            comps = [(lambda tb=tb: proj_v_unit(wv0_t, xt, va0, 0, tb))
                     for tb in range(8, 16)]
            for t4 in range(2, 4):
                comps += [(lambda jc=jc, t4=t4: proj_qk_unit(
                    wqk0_t, xt, bqk_sb[0], qk0, jc, t4)) for jc in range(4)]
            run_iters([it0(0, 0), it0(0, 1), it0(1, 0), it0(1, 1)], comps)
            emit_pred(0, ob0)

            # ============ phase C: attn0 second half + round-1 proj =====
            wv1_t = load_wv(wv1, 1)
            wqk1_t = load_wqk(wqk1, 1)
            va1 = []
            qk1 = qk_tiles(1)
            comps = [resid_unit(0, kc) for kc in range(KC)]
            comps += [(lambda tb=tb: proj_v_unit(wv1_t, xt, va1, 1, tb))
                      for tb in range(8)]
            for t4 in range(2):
                comps += [(lambda jc=jc, t4=t4: proj_qk_unit(
                    wqk1_t, xt, bqk_sb[1], qk1, jc, t4)) for jc in range(4)]
            run_iters([it0(2, 0), it0(2, 1), it0(3, 0), it0(3, 1)], comps)
            emit_pred(1, ob0)

            # ============ phase D: attn1 first half + rest of proj ======
            # prefetch gate-phase pred strips (local read of the AG output)
            pred_v = pred_full[:, :, :].rearrange("h d (s t) -> h d s t",
                                                  s=2)
            predg = []
            for cc in range(KC):
                pg_ = wqk_pool.tile([128, TS], BF16, name=f"predg{cc}",
                                    tag="wqk")
                nc.gpsimd.dma_start(
                    pg_[:], pred_v[bass.ds(hf_sel, 1),
                                   128 * cc:128 * (cc + 1),
                                   bass.ds(st_sel, 1), :]
                    .squeeze(2).squeeze(0))
                predg.append(pg_)

            it1 = lambda q4, hh: (attend_iter, (qk1, va1, bv_sb[1], ob1,
                                                q4, hh))
            comps = [resid_unit(1, kc) for kc in range(KC)]
            comps += [(lambda tb=tb: proj_v_unit(wv1_t, xt, va1, 1, tb))
                      for tb in range(8, 16)]
            for t4 in range(2, 4):
                comps += [(lambda jc=jc, t4=t4: proj_qk_unit(
                    wqk1_t, xt, bqk_sb[1], qk1, jc, t4)) for jc in range(4)]
            run_iters([it1(0, 0), it1(1, 0), it1(0, 1), it1(1, 1)], comps)

            # ============ phase E: attn1 second half + gate pred part ===
            gp_pre = [None] * KC

            def wg_slice(cc, jc):
                return wg_t[cc // 2][:, D * (cc % 2) + 128 * jc:
                                     D * (cc % 2) + 128 * (jc + 1)]

            def inject_gate(jc):
                def f():
                    ps = cps.tile([128, 512], F32, tag="cps", name="gp_ps")
                    for cc in range(KC):
                        nc.tensor.matmul(ps[:], wg_slice(cc, jc),
                                         predg[cc][:],
                                         start=(cc == 0), stop=(cc == KC - 1))
                    t_ = gpre_pool.tile([128, TS], BF16, name=f"gpre{jc}",
                                        tag="gpre")
                    nc.vector.tensor_copy(t_[:], ps[:])
                    gp_pre[jc] = t_
                return f

            comps = [inject_gate(jc) for jc in range(KC)]

            def after_e(i):
                if i == 1:
                    emit_corr(0, ob1[0])
                elif i == 3:
                    emit_corr(1, ob1[1])

            run_iters([it1(2, 0), it1(3, 0), it1(2, 1), it1(3, 1)], comps,
                      after=after_e)

        # ================= phase F: gate + output ==================
        def make_tail(wg_slice=wg_slice, gp_pre=gp_pre, predg=predg):
          def tail(cps_pool):
            gate_tail(nc, tc, sim, wg_slice, gp_pre, predg, wg_t, wo_t,
                      qk_pool, bg_sb, bsel_y, corr_out, y, cps_pool)
          return tail
        pending_tail = make_tail()
    if pending_tail is not None:
        pending_tail(None)


def gate_tail(nc, tc, sim, wg_slice, gp_pre, predg, wg_t, wo_t,
              qk_pool, bg_sb, bsel_y, corr_out, y, cps_pool):
    with ExitStack() as gat:
        gp_pool = gat.enter_context(tc.tile_pool(name="gp", bufs=8))
        y_pool = gat.enter_context(tc.tile_pool(name="yp", bufs=2))
        if cps_pool is None:
            cps_pool = gat.enter_context(tc.tile_pool(name="ps2", bufs=2,
                                                      space="PSUM"))

        # corr strip: channel 128cc belongs to group member cc//2, head
        # pair cc%2 — even chunks land with A2A half 0, odd with half 1;
        # consume in that order so the tail A2A overlaps the gate matmul
        corr_bv = [t[:, :].rearrange("(b r) t -> b r t", b=2)
                   for t in corr_out]
        corr_t = [qk_pool.tile([128, T], BF16, name=f"corrt{i}", tag="qk")
                  for i in range(2)]
        for cc in [0, 2, 4, 6, 1, 3, 5, 7]:
            r0 = 128 * (cc // 2)
            nc.sync.dma_start(
                corr_t[cc // 4][:, 512 * (cc % 4):512 * (cc % 4 + 1)],
                corr_bv[cc % 2][bass.ds(bsel_y, 1),
                                r0:r0 + 128, :].squeeze(0))
        corrg = [corr_t[cc // 4][:, 512 * (cc % 4):512 * (cc % 4 + 1)]
                 for cc in range(KC)]

        pgt = []
        for jc in range(KC):
            ps = cps_pool.tile([128, 512], F32, tag="cps", name=f"ps_g{jc}")
            for i, cc in enumerate([0, 2, 4, 6, 1, 3, 5, 7]):
                nc.tensor.matmul(ps[:], wg_slice(KC + cc, jc), corrg[cc],
                                 start=(i == 0), stop=(i == KC - 1))
            # fold in the pred-half partial computed during attention 1
            nc.vector.tensor_add(ps[:], ps[:], gp_pre[jc][:])
            gt = gp_pool.tile([128, TS], BF16, name=f"gate{jc}", tag="gp")
            nc.scalar.activation(gt[:], ps[:], SIG,
                                 bias=bg_sb[:, jc:jc + 1])
            nc.vector.tensor_mul(gt[:], gt[:], corrg[jc])
            nc.vector.tensor_add(gt[:], gt[:], predg[jc][:])
            pgt.append(gt)

        for tb in range(4):
            for n2 in range(2):
                yt = y_pool.tile([128, 512], F32, tag="y", name="yt")
                ps = cps_pool.tile([128, 512], F32, tag="cps", name="ps_y")
                for cc in range(KC):
                    nc.tensor.matmul(
                        ps[:], pgt[cc][:, 128 * tb:128 * (tb + 1)],
                        wo_t[cc][:, 512 * n2:512 * (n2 + 1)],
                        start=(cc == 0), stop=(cc == KC - 1))
                nc.vector.tensor_copy(yt[:], ps[:])
                nc.sync.dma_start(
                    y[128 * tb:128 * (tb + 1), 512 * n2:512 * (n2 + 1)],
                    yt[:])


_NC = None


def _get_nc():
    global _NC
    if _NC is None:
        _NC = _build()
    return _NC


def make_in_maps(x, Wqkv0, bqkv0, Wqkv1, bqkv1, Wg, bg, Wo, bo):
    bf = mybir.dt.np(BF16)
    mask_np = np.where(np.arange(128)[:, None] > np.arange(128)[None, :],
                       0.0, 1.0).astype(bf)
    ones_np = np.ones((128, HG), bf)
    ones64_np = np.ones((1, 64), bf)
    bg_a = np.ascontiguousarray(bg.reshape(D // 128, 128).T.astype(np.float32))
    wg_np = np.ascontiguousarray(Wg.astype(np.float32).astype(bf))
    wo_np = np.ascontiguousarray(Wo.astype(np.float32).astype(bf))

    in_maps = []
    for c in range(8):
        b, g = divmod(c, G)
        cq = slice(CP * g, CP * (g + 1))
        ck = slice(D + CP * g, D + CP * (g + 1))
        cv = slice(2 * D + CP * g, 2 * D + CP * (g + 1))
        m = {
            "xT": np.ascontiguousarray(x[b].T.astype(np.float32).astype(bf)),
            "mask01": mask_np, "onesc": ones_np, "bg": bg_a,
            "ones64": ones64_np,
            "wg": wg_np, "wo": wo_np,
        }
        for r, (W, bb) in enumerate(((Wqkv0, bqkv0), (Wqkv1, bqkv1))):
            m[f"wqk{r}"] = np.ascontiguousarray(
                np.concatenate([W[:, cq], W[:, ck]], axis=1)
                .astype(np.float32).astype(bf))
            m[f"wv{r}"] = np.ascontiguousarray(
                W[:, cv].astype(np.float32).astype(bf))
            bqk_cat = np.concatenate([bb[cq], bb[ck]]).astype(np.float32)
            m[f"bqk{r}"] = np.ascontiguousarray(bqk_cat.reshape(4, 128).T)
            m[f"bv{r}"] = np.ascontiguousarray(
                bb[cv].astype(np.float32).reshape(2, 128).T)
        in_maps.append(m)
    return in_maps


def assemble(results, bo):
    out = np.empty((B, T, D), np.float32)
    for c in range(8):
        b, g = divmod(c, G)
        out[b, TS * g:TS * (g + 1), :] = results[c]["y"]
    return out + bo.astype(np.float32)


def kernel(x, Wqkv0, bqkv0, Wqkv1, bqkv1, Wg, bg, Wo, bo):
    args = [np.asarray(a) for a in
            (x, Wqkv0, bqkv0, Wqkv1, bqkv1, Wg, bg, Wo, bo)]
    nc = _get_nc()
    in_maps = make_in_maps(*args)
    res = bass_utils.run_bass_kernel_spmd(nc, in_maps, core_ids=list(range(8)))
    return assemble(res.results, args[8])


# revision 4
# speedup vs baseline: 1.0916x; 1.0043x over previous
"""BoostedCausalAttention on 8 trn2 NeuronCores — software-pipelined bf16.

Sharding: core c -> (batch b = c//4, head-group g = c%4, 4 heads each).
Within a 4-core batch group (Megatron-style):
  - qkv projections + attention in "transposed" layout (feature on
    partitions, token on free axis), bf16 matmuls with f32 PSUM accum.
  - The attention inner loop is software-pipelined: the scores matmul for
    key block kb+1 is emitted BEFORE the AV matmul of block kb, so the PE
    never head-of-line blocks on the exp; exp covers both heads of a pair
    in one [128,2,npr] activation.
  - Cross-phase pipelining: round-1 projection interleaves into round-0
    attention's tail; the gate's pred-half matmuls interleave into
    round-1 attention; AllGathers/AllToAlls fire per token-half /
    head-pair so their latency hides under compute.
  - Final output: each core emits y[512 tokens, 1024] f32 (pre-bias);
    host concatenates and adds bo.
"""

from contextlib import ExitStack

import numpy as np

import concourse.bass as bass
import concourse.bacc as bacc
import concourse.tile as tile
import concourse.mybir as mybir
from concourse import bass_utils

B, T, D = 2, 2048, 1024
H, DH = 16, 64
SCALE = DH ** -0.5
G = 4            # head groups (cores per batch)
HG = H // G      # heads per core = 4
CP = HG * DH     # channels per core = 256
TS = T // G      # token slice per core for gate/output phase = 512
KC = D // 128    # contraction chunks over D = 8
TH = T // 2      # token half = 1024

F32 = mybir.dt.float32
BF16 = mybir.dt.bfloat16
EXP = mybir.ActivationFunctionType.Exp
SIG = mybir.ActivationFunctionType.Sigmoid

GROUPS = [[0, 1, 2, 3], [4, 5, 6, 7]]
GROUP8 = [[0, 1, 2, 3, 4, 5, 6, 7]]


def _build(sim=False, reps=1):
    nc = bacc.Bacc("TRN2", target_bir_lowering=False, debug=False, num_devices=8)

    xT = nc.dram_tensor("xT", [D, T], BF16, kind="ExternalInput")
    wqk0 = nc.dram_tensor("wqk0", [D, 2 * CP], BF16, kind="ExternalInput")
    wv0 = nc.dram_tensor("wv0", [D, CP], BF16, kind="ExternalInput")
    bqk0 = nc.dram_tensor("bqk0", [128, 4], F32, kind="ExternalInput")
    bv0 = nc.dram_tensor("bv0", [128, 2], F32, kind="ExternalInput")
    wqk1 = nc.dram_tensor("wqk1", [D, 2 * CP], BF16, kind="ExternalInput")
    wv1 = nc.dram_tensor("wv1", [D, CP], BF16, kind="ExternalInput")
    bqk1 = nc.dram_tensor("bqk1", [128, 4], F32, kind="ExternalInput")
    bv1 = nc.dram_tensor("bv1", [128, 2], F32, kind="ExternalInput")
    wg = nc.dram_tensor("wg", [2 * D, D], BF16, kind="ExternalInput")
    bg = nc.dram_tensor("bg", [128, D // 128], F32, kind="ExternalInput")
    wo = nc.dram_tensor("wo", [D, D], BF16, kind="ExternalInput")
    mask01 = nc.dram_tensor("mask01", [128, 128], BF16, kind="ExternalInput")
    onesc = nc.dram_tensor("onesc", [128, HG], BF16, kind="ExternalInput")
    ones64 = nc.dram_tensor("ones64", [1, 64], BF16, kind="ExternalInput")
    y = nc.dram_tensor("y", [TS, D], F32, kind="ExternalOutput")

    with tile.TileContext(nc) as tc, ExitStack() as glb:
        consts = glb.enter_context(tc.tile_pool(name="consts", bufs=1))
        # 4KB-per-partition slots: x^T/residual tiles, later reused by Wg
        big8 = glb.enter_context(tc.tile_pool(name="big8", bufs=8))
        dpool = glb.enter_context(tc.tile_pool(name="dpool", bufs=1, space="DRAM"))

        # ---- constants ----
        bqk_sb = [consts.tile([128, 4], F32, name=f"bqk_sb{r}") for r in range(2)]
        bv_sb = [consts.tile([128, 2], F32, name=f"bv_sb{r}") for r in range(2)]
        bg_sb = consts.tile([128, D // 128], F32)
        mask_sb = consts.tile([128, 128], BF16)
        ones_sb = consts.tile([128, HG], BF16)
        ones64_sb = consts.tile([1, 64], BF16)
        nc.gpsimd.dma_start(ones_sb[:], onesc[:, :])
        nc.gpsimd.dma_start(ones64_sb[:], ones64[:, :])
        nc.gpsimd.dma_start(bqk_sb[0][:], bqk0[:, :])
        nc.gpsimd.dma_start(bqk_sb[1][:], bqk1[:, :])
        nc.gpsimd.dma_start(bv_sb[0][:], bv0[:, :])
        nc.gpsimd.dma_start(bv_sb[1][:], bv1[:, :])
        nc.gpsimd.dma_start(bg_sb[:], bg[:, :])
        nc.gpsimd.dma_start(mask_sb[:], mask01[:, :])

        # ---- collective DRAM buffers ----
        pred_part = [dpool.tile([CP, TH], BF16, name=f"pred_part{i}")
                     for i in range(2)]
        pred_full = dpool.tile([2, D, TH], BF16, name="pred_full")
        corr_a2a = [dpool.tile([8 * 128, TS], BF16, name=f"corr_a2a{i}")
                    for i in range(2)]
        corr_out = [dpool.tile([8 * 128, TS], BF16, name=f"corr_out{i}")
                    for i in range(2)]

        wqk_pool = glb.enter_context(tc.tile_pool(name="wqk", bufs=2 * KC))
        qk_pool = glb.enter_context(tc.tile_pool(name="qk", bufs=9))
        wo_pool = glb.enter_context(tc.tile_pool(name="wo", bufs=1))
        wg_pool = glb.enter_context(tc.tile_pool(name="wg", bufs=1))
        gpre_pool = glb.enter_context(tc.tile_pool(name="gpre", bufs=KC))

        # gate/output weights are SBUF-resident for the kernel's lifetime:
        # loaded once, so the xt pool slots never serialize through them
        wg_t = []
        for i in range(KC):
            t_ = wg_pool.tile([128, 2 * D], BF16, name=f"wg{i}")
            nc.gpsimd.dma_start(
                t_[:].rearrange("p (a d) -> p a d", a=2),
                bass.AP(tensor=wg, offset=256 * i * D,
                        ap=[[D, 128], [128 * D, 2], [1, D]]))
            wg_t.append(t_)
        wo_t = []
        for cc in range(KC):
            t_ = wo_pool.tile([128, D], BF16, name=f"wo{cc}")
            eng = nc.scalar if cc % 2 else nc.sync
            eng.dma_start(t_[:], wo[128 * cc:128 * (cc + 1), :])
            wo_t.append(t_)

        # strip-select registers (snap donates in place); set up once.
        # g = pid%4: half hf = g//2, within-half slot st = g%2.
        pid = nc.gpsimd.partition_id()
        r_t = nc.gpsimd.alloc_register("selt")
        nc.gpsimd.reg_div(r_t, pid, 2)
        half_sel = nc.gpsimd.snap(r_t, donate=True, min_val=0, max_val=3)
        r_hf = nc.gpsimd.alloc_register("selhf")
        nc.gpsimd.reg_mod(r_hf, half_sel, 2)
        hf_sel = nc.gpsimd.snap(r_hf, donate=True, min_val=0, max_val=1)
        r_st = nc.gpsimd.alloc_register("selst")
        nc.gpsimd.reg_mod(r_st, pid, 2)
        st_sel = nc.gpsimd.snap(r_st, donate=True, min_val=0, max_val=1)

        pid_y = nc.sync.partition_id()
        r_by = nc.sync.alloc_register("selby")
        nc.sync.reg_div(r_by, pid_y, G)
        bsel_y = nc.sync.snap(r_by, donate=True, min_val=0, max_val=1)

        _body(nc, tc, sim, reps, big8, wqk_pool, qk_pool, wg_t, wo_t, gpre_pool,
              bqk_sb, bv_sb, bg_sb, mask_sb, ones_sb, ones64_sb,
              hf_sel, st_sel, bsel_y, xT, wqk0, wv0, wqk1, wv1, y,
              pred_part, pred_full, corr_a2a, corr_out)

    nc.compile()
    return nc


def _body(nc, tc, sim, reps, big8, wqk_pool, qk_pool, wg_t, wo_t, gpre_pool,
          bqk_sb, bv_sb, bg_sb, mask_sb, ones_sb, ones64_sb,
          hf_sel, st_sel, bsel_y, xT, wqk0, wv0, wqk1, wv1, y,
          pred_part, pred_full, corr_a2a, corr_out):
    pending_tail = None
    for _rep in range(reps):
        xt = []
        for kc in range(KC):
            xt.append(big8.tile([128, T], BF16, name=f"xt{kc}", tag="b8"))
        with ExitStack() as att:
            wv_pool = att.enter_context(tc.tile_pool(name="wv", bufs=KC))
            vaug_pool = att.enter_context(tc.tile_pool(name="vaug", bufs=24))
            p_pool = att.enter_context(tc.tile_pool(name="pp", bufs=4))
            ob_pool = att.enter_context(tc.tile_pool(name="ob", bufs=4))
            o_pool = att.enter_context(tc.tile_pool(name="op", bufs=3))
            bc_pool = att.enter_context(tc.tile_pool(name="bc", bufs=2))
            sm_pool = att.enter_context(tc.tile_pool(name="sm", bufs=2))
            # PSUM: 2x scores staging (2 banks each) + 2x av + 2x shared
            # companion/bps slots = 8 banks
            psum = att.enter_context(tc.tile_pool(name="psum", bufs=2,
                                                  space="PSUM"))
            avps = att.enter_context(tc.tile_pool(name="avps", bufs=2,
                                                  space="PSUM"))
            cps = att.enter_context(tc.tile_pool(name="cps", bufs=2,
                                                 space="PSUM"))

            def load_wqk(wqk_d, rnd):
                wt = []
                for kc in range(KC):
                    t_ = wqk_pool.tile([128, 2 * CP], BF16,
                                       name=f"wqk{rnd}_{kc}", tag="wqk")
                    eng = nc.scalar if kc % 2 else nc.sync
                    eng.dma_start(t_[:], wqk_d[128 * kc:128 * (kc + 1), :])
                    wt.append(t_)
                return wt

            def load_wv(wv_d, rnd):
                wt = []
                for kc in range(KC):
                    t_ = wv_pool.tile([128, CP], BF16,
                                      name=f"wv{rnd}_{kc}", tag="wv")
                    eng = nc.scalar if kc % 2 else nc.sync
                    eng.dma_start(t_[:], wv_d[128 * kc:128 * (kc + 1), :])
                    wt.append(t_)
                return wt

            def qk_tiles(rnd):
                return [qk_pool.tile([128, T], BF16, name=f"qk{rnd}_{jc}",
                                     tag="qk") for jc in range(4)]

            def proj_qk_unit(wt, src, biasc, qk, jc, t4):
                """One [128,512] q^T/k^T block: 8 matmuls + bias add."""
                ps = cps.tile([128, 512], F32, tag="cps", name="ps_pqk")
                for kc in range(KC):
                    nc.tensor.matmul(
                        ps[:], wt[kc][:, 128 * jc:128 * (jc + 1)],
                        src[kc][:, 512 * t4:512 * (t4 + 1)],
                        start=(kc == 0), stop=(kc == KC - 1))
                nc.vector.tensor_scalar_add(
                    qk[jc][:, 512 * t4:512 * (t4 + 1)], ps[:],
                    biasc[:, jc:jc + 1])

            def proj_v_unit(wt, src, va, rnd, tb):
                """One 128-token block of v (natural layout + ones col)."""
                t_ = vaug_pool.tile([128, HG, DH + 1], BF16,
                                    name=f"va{rnd}_{tb}", tag="va")
                ps = cps.tile([128, 512], F32, tag="cps", name="ps_pv")
                for kc in range(KC):
                    nc.tensor.matmul(
                        ps[:, 0:CP], src[kc][:, 128 * tb:128 * (tb + 1)],
                        wt[kc][:], start=(kc == 0), stop=(kc == KC - 1))
                nc.vector.tensor_copy(
                    t_[:, :, 0:DH],
                    ps[:, 0:CP].rearrange("p (h d) -> p h d", h=HG))
                nc.vector.tensor_copy(t_[:, :, DH:DH + 1],
                                      ones_sb[:, :, None])
                va.append(t_)

            def attend_iter(qk, va, biasv, ob, q4, hh):
                """One (q4, hh) attention iteration, software-pipelined:
                scores+exp run one key block ahead of the AV matmuls."""
                nblk = 4 * (q4 + 1)
                qt, kt = qk[hh], qk[2 + hh]
                av = [avps.tile([DH + 1, 512], F32, tag="av",
                                name=f"av{h2}") for h2 in range(2)]
                pending = []

                def emit_scores(kb):
                    diag = kb - 4 * q4
                    c0 = max(0, 128 * diag)
                    npr = 512 - c0
                    ps = psum.tile([128, 2, 512], F32, tag="ps", name="s2")
                    for h2 in range(2):
                        base = 64 * h2
                        nc.tensor.matmul(
                            ps[:, h2, 0:npr],
                            kt[base:base + 64, 128 * kb:128 * (kb + 1)],
                            qt[base:base + 64,
                               512 * q4 + c0:512 * (q4 + 1)],
                            start=True, stop=True)
                    p = p_pool.tile([128, 2, 512], BF16, tag="p", name="p2")
                    nc.scalar.activation(p[:, :, 0:npr], ps[:, :, 0:npr],
                                         EXP, scale=SCALE)
                    if diag >= 0:
                        for h2 in range(2):
                            nc.vector.tensor_mul(
                                p[:, h2, 0:128], p[:, h2, 0:128], mask_sb[:])
                    pending.append((p, kb, c0, npr))

                def emit_av():
                    p, kb, c0, npr = pending.pop(0)
                    for h2 in range(2):
                        h = 2 * hh + h2
                        nc.tensor.matmul(
                            av[h2][:, c0:512], va[kb][:, h, :],
                            p[:, h2, 0:npr],
                            start=(kb == 0), stop=(kb == nblk - 1))

                for kb in range(nblk):
                    emit_scores(kb)
                    if len(pending) == 3:
                        emit_av()
                while pending:
                    emit_av()

                recr = sm_pool.tile([1, 1024], BF16, tag="recr", name="recr")
                with nc.allow_low_precision(
                        reason="softmax recip rounds to bf16"):
                    nc.vector.reciprocal(recr[0:1, 0:512],
                                         av[0][DH:DH + 1, :])
                    nc.vector.reciprocal(recr[0:1, 512:1024],
                                         av[1][DH:DH + 1, :])
                bps = cps.tile([128, 512], F32, tag="cps", name="bps")
                for h2 in range(2):
                    nc.tensor.matmul(
                        bps[64 * h2:64 * (h2 + 1), :], ones64_sb[:],
                        recr[0:1, 512 * h2:512 * (h2 + 1)],
                        start=True, stop=True)
                bc = bc_pool.tile([128, 512], F32, tag="bc", name="bc")
                nc.vector.tensor_copy(bc[:], bps[:])
                osl = ob[hh][:, 512 * q4:512 * (q4 + 1)]
                for h2 in range(2):
                    nc.vector.tensor_mul(
                        osl[64 * h2:64 * (h2 + 1), :],
                        av[h2][0:DH, :],
                        bc[64 * h2:64 * (h2 + 1), :])
                nc.vector.tensor_scalar_add(osl, osl, biasv[:, hh:hh + 1])

            def run_iters(iters, comps, after=None):
                """Emit attention iterations with companion units spread
                between them (fills the PE while the Act engine works)."""
                comps = list(comps)
                n = len(iters)
                for i, (fn, args) in enumerate(iters):
                    fn(*args)
                    left = n - i - 1
                    if comps:
                        take = (len(comps) if left == 0
                                else max(1, len(comps) // (left + 1)))
                        for _ in range(take):
                            if comps:
                                comps.pop(0)()
                    if after is not None:
                        after(i)
                for c in comps:
                    c()

            def emit_pred(half, ob):
                cs = slice(TH * half, TH * (half + 1))
                for hh in range(2):
                    nc.sync.dma_start(
                        pred_part[half][128 * hh:128 * (hh + 1), :],
                        ob[hh][:, cs])
                if sim:
                    nc.sync.dma_start(pred_full[half, 0:CP, :],
                                      pred_part[half][:, :])
                else:
                    nc.gpsimd.collective_compute(
                        "AllGather", mybir.AluOpType.bypass,
                        replica_groups=GROUPS,
                        ins=[pred_part[half][:, :]],
                        outs=[pred_full[half, :, :]])

            def emit_corr(hh, obh):
                for s in range(G):
                    for dup in range(2):
                        c_ = 4 * dup + s
                        nc.sync.dma_start(
                            corr_a2a[hh][128 * c_:128 * (c_ + 1), :],
                            obh[:, 512 * s:512 * (s + 1)])
                if sim:
                    nc.sync.dma_start(corr_out[hh][0:128, 0:64],
                                      corr_a2a[hh][0:128, 0:64])
                else:
                    nc.gpsimd.collective_compute(
                        "AllToAll", mybir.AluOpType.bypass,
                        replica_groups=GROUP8,
                        ins=[corr_a2a[hh][:, :]],
                        outs=[corr_out[hh][:, :]])

            def resid_unit(hf, kc):
                def f():
                    cs = slice(TH * hf, TH * (hf + 1))
                    pt = o_pool.tile([128, TH], BF16, tag="op", name="predld")
                    nc.sync.dma_start(
                        pt[:], pred_full[hf, 128 * kc:128 * (kc + 1), :])
                    nc.vector.tensor_sub(xt[kc][:, cs], xt[kc][:, cs], pt[:])
                return f

            # ================= phase A: round-0 projection ==============
            wv0_t = load_wv(wv0, 0)
            for hf in range(4):
                for kc in range(KC):
                    eng = nc.scalar if (kc + hf) % 2 else nc.sync
                    eng.dma_start(xt[kc][:, 512 * hf:512 * (hf + 1)],
                                  xT[128 * kc:128 * (kc + 1),
                                     512 * hf:512 * (hf + 1)])
                if hf == 0:
                    wqk0_t = load_wqk(wqk0, 0)
            va0 = []
            qk0 = qk_tiles(0)
            # emit only what attn0's first token half needs, the rest
            # becomes companion work under attention
            for tb in range(8):
                proj_v_unit(wv0_t, xt, va0, 0, tb)
            for t4 in range(2):
                for jc in range(4):
                    proj_qk_unit(wqk0_t, xt, bqk_sb[0], qk0, jc, t4)

            # previous rep's gate/output tail: emitted here so its corr
            # A2A latency and PE tail overlap this rep's projection start
            if pending_tail is not None:
                pending_tail(cps)
                pending_tail = None

            ob0 = [ob_pool.tile([128, T], BF16, tag="ob", name=f"ob0_{hh}")
                   for hh in range(2)]
            ob1 = [ob_pool.tile([128, T], BF16, tag="ob", name=f"ob1_{hh}")
                   for hh in range(2)]

            # ============ phase B: attn0 first token half ===============
            it0 = lambda q4, hh: (attend_iter, (qk0, va0, bv_sb[0], ob0,
                                                q4, hh))
            comps = [(lambda tb=tb: proj_v_unit(wv0_t, xt, va0, 0, tb))
                     for tb in range(8, 16)]
            for t4 in range(2, 4):
                comps += [(lambda jc=jc, t4=t4: proj_qk_unit(
                    wqk0_t, xt, bqk_sb[0], qk0, jc, t4)) for jc in range(4)]
            run_iters([it0(0, 0), it0(0, 1), it0(1, 0), it0(1, 1)], comps)
            emit_pred(0, ob0)

            # ============ phase C: attn0 second half + round-1 proj =====
            wv1_t = load_wv(wv1, 1)
            wqk1_t = load_wqk(wqk1, 1)
            va1 = []
            qk1 = qk_tiles(1)
            comps = [resid_unit(0, kc) for kc in range(KC)]
            comps += [(lambda tb=tb: proj_v_unit(wv1_t, xt, va1, 1, tb))
                      for tb in range(8)]
            for t4 in range(2):
                comps += [(lambda jc=jc, t4=t4: proj_qk_unit(
                    wqk1_t, xt, bqk_sb[1], qk1, jc, t4)) for jc in range(4)]
            run_iters([it0(2, 0), it0(2, 1), it0(3, 0), it0(3, 1)], comps)
            emit_pred(1, ob0)

            # ============ phase D: attn1 first half + rest of proj ======
            # prefetch gate-phase pred strips (local read of the AG output)
            pred_v = pred_full[:, :, :].rearrange("h d (s t) -> h d s t",
                                                  s=2)
            predg = []
            for cc in range(KC):
                pg_ = wqk_pool.tile([128, TS], BF16, name=f"predg{cc}",
                                    tag="wqk")
                nc.gpsimd.dma_start(
                    pg_[:], pred_v[bass.ds(hf_sel, 1),
                                   128 * cc:128 * (cc + 1),
                                   bass.ds(st_sel, 1), :]
                    .squeeze(2).squeeze(0))
                predg.append(pg_)

            it1 = lambda q4, hh: (attend_iter, (qk1, va1, bv_sb[1], ob1,
                                                q4, hh))
            comps = [resid_unit(1, kc) for kc in range(KC)]
            comps += [(lambda tb=tb: proj_v_unit(wv1_t, xt, va1, 1, tb))
                      for tb in range(8, 16)]
            for t4 in range(2, 4):
                comps += [(lambda jc=jc, t4=t4: proj_qk_unit(
                    wqk1_t, xt, bqk_sb[1], qk1, jc, t4)) for jc in range(4)]
            run_iters([it1(0, 0), it1(1, 0), it1(0, 1), it1(1, 1)], comps)

            # ============ phase E: attn1 second half + gate pred part ===
            gp_pre = [None] * KC

            def wg_slice(cc, jc):
                return wg_t[cc // 2][:, D * (cc % 2) + 128 * jc:
                                     D * (cc % 2) + 128 * (jc + 1)]

            def inject_gate(jc):
                def f():
                    ps = cps.tile([128, 512], F32, tag="cps", name="gp_ps")
                    for cc in range(KC):
                        nc.tensor.matmul(ps[:], wg_slice(cc, jc),
                                         predg[cc][:],
                                         start=(cc == 0), stop=(cc == KC - 1))
                    t_ = gpre_pool.tile([128, TS], BF16, name=f"gpre{jc}",
                                        tag="gpre")
                    nc.vector.tensor_copy(t_[:], ps[:])
                    gp_pre[jc] = t_
                return f

            comps = [inject_gate(jc) for jc in range(KC)]

            def after_e(i):
                if i == 1:
                    emit_corr(0, ob1[0])
                elif i == 3:
                    emit_corr(1, ob1[1])

            run_iters([it1(2, 0), it1(3, 0), it1(2, 1), it1(3, 1)], comps,
                      after=after_e)

        # ================= phase F: gate + output ==================
        def make_tail(wg_slice=wg_slice, gp_pre=gp_pre, predg=predg):
          def tail(cps_pool):
            gate_tail(nc, tc, sim, wg_slice, gp_pre, predg, wg_t, wo_t,
                      qk_pool, bg_sb, bsel_y, corr_out, y, cps_pool)
          return tail
        pending_tail = make_tail()
    if pending_tail is not None:
        pending_tail(None)


def gate_tail(nc, tc, sim, wg_slice, gp_pre, predg, wg_t, wo_t,
              qk_pool, bg_sb, bsel_y, corr_out, y, cps_pool):
    with ExitStack() as gat:
        gp_pool = gat.enter_context(tc.tile_pool(name="gp", bufs=8))
        y_pool = gat.enter_context(tc.tile_pool(name="yp", bufs=2))
        if cps_pool is None:
            cps_pool = gat.enter_context(tc.tile_pool(name="ps2", bufs=2,
                                                      space="PSUM"))

        # corr strip: channel 128cc belongs to group member cc//2, head
        # pair cc%2 — even chunks land with A2A half 0, odd with half 1;
        # consume in that order so the tail A2A overlaps the gate matmul
        corr_bv = [t[:, :].rearrange("(b r) t -> b r t", b=2)
                   for t in corr_out]
        corr_t = [qk_pool.tile([128, T], BF16, name=f"corrt{i}", tag="qk")
                  for i in range(2)]
        for cc in [0, 2, 4, 6, 1, 3, 5, 7]:
            r0 = 128 * (cc // 2)
            nc.sync.dma_start(
                corr_t[cc // 4][:, 512 * (cc % 4):512 * (cc % 4 + 1)],
                corr_bv[cc % 2][bass.ds(bsel_y, 1),
                                r0:r0 + 128, :].squeeze(0))
        corrg = [corr_t[cc // 4][:, 512 * (cc % 4):512 * (cc % 4 + 1)]
                 for cc in range(KC)]

        pgt = []
        for jc in range(KC):
            ps = cps_pool.tile([128, 512], F32, tag="cps", name=f"ps_g{jc}")
            for i, cc in enumerate([0, 2, 4, 6, 1, 3, 5, 7]):
                nc.tensor.matmul(ps[:], wg_slice(KC + cc, jc), corrg[cc],
                                 start=(i == 0), stop=(i == KC - 1))
            # fold in the pred-half partial computed during attention 1
            nc.vector.tensor_add(ps[:], ps[:], gp_pre[jc][:])
            gt = gp_pool.tile([128, TS], BF16, name=f"gate{jc}", tag="gp")
            nc.scalar.activation(gt[:], ps[:], SIG,
                                 bias=bg_sb[:, jc:jc + 1])
            nc.vector.tensor_mul(gt[:], gt[:], corrg[jc])
            nc.vector.tensor_add(gt[:], gt[:], predg[jc][:])
            pgt.append(gt)

        for tb in range(4):
            for n2 in range(2):
                yt = y_pool.tile([128, 512], F32, tag="y", name="yt")
                ps = cps_pool.tile([128, 512], F32, tag="cps", name="ps_y")
                for cc in range(KC):
                    nc.tensor.matmul(
                        ps[:], pgt[cc][:, 128 * tb:128 * (tb + 1)],
                        wo_t[cc][:, 512 * n2:512 * (n2 + 1)],
                        start=(cc == 0), stop=(cc == KC - 1))
                nc.vector.tensor_copy(yt[:], ps[:])
                nc.sync.dma_start(
                    y[128 * tb:128 * (tb + 1), 512 * n2:512 * (n2 + 1)],
                    yt[:])


_NC = None


def _get_nc():
    global _NC
    if _NC is None:
        _NC = _build()
    return _NC


def make_in_maps(x, Wqkv0, bqkv0, Wqkv1, bqkv1, Wg, bg, Wo, bo):
    bf = mybir.dt.np(BF16)
    mask_np = np.where(np.arange(128)[:, None] > np.arange(128)[None, :],
                       0.0, 1.0).astype(bf)
    ones_np = np.ones((128, HG), bf)
    ones64_np = np.ones((1, 64), bf)
    bg_a = np.ascontiguousarray(bg.reshape(D // 128, 128).T.astype(np.float32))
    wg_np = np.ascontiguousarray(Wg.astype(np.float32).astype(bf))
    wo_np = np.ascontiguousarray(Wo.astype(np.float32).astype(bf))

    in_maps = []
    for c in range(8):
        b, g = divmod(c, G)
        cq = slice(CP * g, CP * (g + 1))
        ck = slice(D + CP * g, D + CP * (g + 1))
        cv = slice(2 * D + CP * g, 2 * D + CP * (g + 1))
        m = {
            "xT": np.ascontiguousarray(x[b].T.astype(np.float32).astype(bf)),
            "mask01": mask_np, "onesc": ones_np, "bg": bg_a,
            "ones64": ones64_np,
            "wg": wg_np, "wo": wo_np,
        }
        for r, (W, bb) in enumerate(((Wqkv0, bqkv0), (Wqkv1, bqkv1))):
            m[f"wqk{r}"] = np.ascontiguousarray(
                np.concatenate([W[:, cq], W[:, ck]], axis=1)
                .astype(np.float32).astype(bf))
            m[f"wv{r}"] = np.ascontiguousarray(
                W[:, cv].astype(np.float32).astype(bf))
            bqk_cat = np.concatenate([bb[cq], bb[ck]]).astype(np.float32)
            m[f"bqk{r}"] = np.ascontiguousarray(bqk_cat.reshape(4, 128).T)
            m[f"bv{r}"] = np.ascontiguousarray(
                bb[cv].astype(np.float32).reshape(2, 128).T)
        in_maps.append(m)
    return in_maps


def assemble(results, bo):
    out = np.empty((B, T, D), np.float32)
    for c in range(8):
        b, g = divmod(c, G)
        out[b, TS * g:TS * (g + 1), :] = results[c]["y"]
    return out + bo.astype(np.float32)


def kernel(x, Wqkv0, bqkv0, Wqkv1, bqkv1, Wg, bg, Wo, bo):
    args = [np.asarray(a) for a in
            (x, Wqkv0, bqkv0, Wqkv1, bqkv1, Wg, bg, Wo, bo)]
    nc = _get_nc()
    in_maps = make_in_maps(*args)
    res = bass_utils.run_bass_kernel_spmd(nc, in_maps, core_ids=list(range(8)))
    return assemble(res.results, args[8])


# revision 5
# speedup vs baseline: 1.0985x; 1.0063x over previous
"""BoostedCausalAttention on 8 trn2 NeuronCores — software-pipelined bf16.

Sharding: core c -> (batch b = c//4, head-group g = c%4, 4 heads each).
Within a 4-core batch group (Megatron-style):
  - qkv projections + attention in "transposed" layout (feature on
    partitions, token on free axis), bf16 matmuls with f32 PSUM accum.
  - The attention inner loop is software-pipelined: the scores matmul for
    key block kb+1 is emitted BEFORE the AV matmul of block kb, so the PE
    never head-of-line blocks on the exp; exp covers both heads of a pair
    in one [128,2,npr] activation.
  - Cross-phase pipelining: round-1 projection interleaves into round-0
    attention's tail; the gate's pred-half matmuls interleave into
    round-1 attention; AllGathers/AllToAlls fire per token-half /
    head-pair so their latency hides under compute.
  - Final output: each core emits y[512 tokens, 1024] f32 (pre-bias);
    host concatenates and adds bo.
"""

from contextlib import ExitStack

import numpy as np

import concourse.bass as bass
import concourse.bacc as bacc
import concourse.tile as tile
import concourse.mybir as mybir
from concourse import bass_utils

B, T, D = 2, 2048, 1024
H, DH = 16, 64
SCALE = DH ** -0.5
G = 4            # head groups (cores per batch)
HG = H // G      # heads per core = 4
CP = HG * DH     # channels per core = 256
TS = T // G      # token slice per core for gate/output phase = 512
KC = D // 128    # contraction chunks over D = 8
TH = T // 2      # token half = 1024

F32 = mybir.dt.float32
BF16 = mybir.dt.bfloat16
EXP = mybir.ActivationFunctionType.Exp
SIG = mybir.ActivationFunctionType.Sigmoid

GROUPS = [[0, 1, 2, 3], [4, 5, 6, 7]]
GROUP8 = [[0, 1, 2, 3, 4, 5, 6, 7]]


def _build(sim=False, reps=1):
    nc = bacc.Bacc("TRN2", target_bir_lowering=False, debug=False, num_devices=8)

    xT = nc.dram_tensor("xT", [D, T], BF16, kind="ExternalInput")
    wqk0 = nc.dram_tensor("wqk0", [D, 2 * CP], BF16, kind="ExternalInput")
    wv0 = nc.dram_tensor("wv0", [D, CP], BF16, kind="ExternalInput")
    bqk0 = nc.dram_tensor("bqk0", [128, 4], F32, kind="ExternalInput")
    bv0 = nc.dram_tensor("bv0", [128, 2], F32, kind="ExternalInput")
    wqk1 = nc.dram_tensor("wqk1", [D, 2 * CP], BF16, kind="ExternalInput")
    wv1 = nc.dram_tensor("wv1", [D, CP], BF16, kind="ExternalInput")
    bqk1 = nc.dram_tensor("bqk1", [128, 4], F32, kind="ExternalInput")
    bv1 = nc.dram_tensor("bv1", [128, 2], F32, kind="ExternalInput")
    wg = nc.dram_tensor("wg", [2 * D, D], BF16, kind="ExternalInput")
    bg = nc.dram_tensor("bg", [128, D // 128], F32, kind="ExternalInput")
    wo = nc.dram_tensor("wo", [D, D], BF16, kind="ExternalInput")
    mask01 = nc.dram_tensor("mask01", [128, 128], BF16, kind="ExternalInput")
    onesc = nc.dram_tensor("onesc", [128, HG], BF16, kind="ExternalInput")
    ones64 = nc.dram_tensor("ones64", [1, 64], BF16, kind="ExternalInput")
    y = nc.dram_tensor("y", [TS, D], F32, kind="ExternalOutput")

    with tile.TileContext(nc) as tc, ExitStack() as glb:
        consts = glb.enter_context(tc.tile_pool(name="consts", bufs=1))
        # 4KB-per-partition slots: x^T/residual tiles, later reused by Wg
        big8 = glb.enter_context(tc.tile_pool(name="big8", bufs=8))
        dpool = glb.enter_context(tc.tile_pool(name="dpool", bufs=1, space="DRAM"))

        # ---- constants ----
        bqk_sb = [consts.tile([128, 4], F32, name=f"bqk_sb{r}") for r in range(2)]
        bv_sb = [consts.tile([128, 2], F32, name=f"bv_sb{r}") for r in range(2)]
        bg_sb = consts.tile([128, D // 128], F32)
        mask_sb = consts.tile([128, 128], BF16)
        ones_sb = consts.tile([128, HG], BF16)
        ones64_sb = consts.tile([1, 64], BF16)
        nc.gpsimd.dma_start(ones_sb[:], onesc[:, :])
        nc.gpsimd.dma_start(ones64_sb[:], ones64[:, :])
        nc.gpsimd.dma_start(bqk_sb[0][:], bqk0[:, :])
        nc.gpsimd.dma_start(bqk_sb[1][:], bqk1[:, :])
        nc.gpsimd.dma_start(bv_sb[0][:], bv0[:, :])
        nc.gpsimd.dma_start(bv_sb[1][:], bv1[:, :])
        nc.gpsimd.dma_start(bg_sb[:], bg[:, :])
        nc.gpsimd.dma_start(mask_sb[:], mask01[:, :])

        # ---- collective DRAM buffers ----
        pred_part = [dpool.tile([CP, TH], BF16, name=f"pred_part{i}")
                     for i in range(2)]
        pred_full = dpool.tile([2, D, TH], BF16, name="pred_full")
        corr_a2a = [dpool.tile([8 * 128, TS], BF16, name=f"corr_a2a{i}")
                    for i in range(2)]
        corr_out = [dpool.tile([8 * 128, TS], BF16, name=f"corr_out{i}")
                    for i in range(2)]

        wqk_pool = glb.enter_context(tc.tile_pool(name="wqk", bufs=2 * KC))
        qk_pool = glb.enter_context(tc.tile_pool(name="qk", bufs=9))
        wo_pool = glb.enter_context(tc.tile_pool(name="wo", bufs=1))
        wg_pool = glb.enter_context(tc.tile_pool(name="wg", bufs=1))
        gpre_pool = glb.enter_context(tc.tile_pool(name="gpre", bufs=KC))

        # gate/output weights are SBUF-resident for the kernel's lifetime:
        # loaded once, so the xt pool slots never serialize through them
        wg_t = []
        for i in range(KC):
            t_ = wg_pool.tile([128, 2 * D], BF16, name=f"wg{i}")
            nc.gpsimd.dma_start(
                t_[:].rearrange("p (a d) -> p a d", a=2),
                bass.AP(tensor=wg, offset=256 * i * D,
                        ap=[[D, 128], [128 * D, 2], [1, D]]))
            wg_t.append(t_)
        wo_t = []
        for cc in range(KC):
            t_ = wo_pool.tile([128, D], BF16, name=f"wo{cc}")
            eng = nc.scalar if cc % 2 else nc.sync
            eng.dma_start(t_[:], wo[128 * cc:128 * (cc + 1), :])
            wo_t.append(t_)

        # strip-select registers (snap donates in place); set up once.
        # g = pid%4: half hf = g//2, within-half slot st = g%2.
        pid = nc.gpsimd.partition_id()
        r_t = nc.gpsimd.alloc_register("selt")
        nc.gpsimd.reg_div(r_t, pid, 2)
        half_sel = nc.gpsimd.snap(r_t, donate=True, min_val=0, max_val=3)
        r_hf = nc.gpsimd.alloc_register("selhf")
        nc.gpsimd.reg_mod(r_hf, half_sel, 2)
        hf_sel = nc.gpsimd.snap(r_hf, donate=True, min_val=0, max_val=1)
        r_st = nc.gpsimd.alloc_register("selst")
        nc.gpsimd.reg_mod(r_st, pid, 2)
        st_sel = nc.gpsimd.snap(r_st, donate=True, min_val=0, max_val=1)

        pid_y = nc.sync.partition_id()
        r_by = nc.sync.alloc_register("selby")
        nc.sync.reg_div(r_by, pid_y, G)
        bsel_y = nc.sync.snap(r_by, donate=True, min_val=0, max_val=1)

        _body(nc, tc, sim, reps, big8, wqk_pool, qk_pool, wg_t, wo_t, gpre_pool,
              bqk_sb, bv_sb, bg_sb, mask_sb, ones_sb, ones64_sb,
              hf_sel, st_sel, bsel_y, xT, wqk0, wv0, wqk1, wv1, y,
              pred_part, pred_full, corr_a2a, corr_out)

    nc.compile()
    return nc


def _body(nc, tc, sim, reps, big8, wqk_pool, qk_pool, wg_t, wo_t, gpre_pool,
          bqk_sb, bv_sb, bg_sb, mask_sb, ones_sb, ones64_sb,
          hf_sel, st_sel, bsel_y, xT, wqk0, wv0, wqk1, wv1, y,
          pred_part, pred_full, corr_a2a, corr_out):
    pending_tail = None
    for _rep in range(reps):
        xt = []
        for kc in range(KC):
            xt.append(big8.tile([128, T], BF16, name=f"xt{kc}", tag="b8"))
        with ExitStack() as att:
            wv_pool = att.enter_context(tc.tile_pool(name="wv", bufs=KC))
            vaug_pool = att.enter_context(tc.tile_pool(name="vaug", bufs=23))
            p_pool = att.enter_context(tc.tile_pool(name="pp", bufs=5))
            ob_pool = att.enter_context(tc.tile_pool(name="ob", bufs=4))
            o_pool = att.enter_context(tc.tile_pool(name="op", bufs=3))
            bc_pool = att.enter_context(tc.tile_pool(name="bc", bufs=2))
            sm_pool = att.enter_context(tc.tile_pool(name="sm", bufs=1))
            # PSUM: 2x scores staging (2 banks each) + 2x av + 2x shared
            # companion/bps slots = 8 banks
            psum = att.enter_context(tc.tile_pool(name="psum", bufs=2,
                                                  space="PSUM"))
            avps = att.enter_context(tc.tile_pool(name="avps", bufs=2,
                                                  space="PSUM"))
            cps = att.enter_context(tc.tile_pool(name="cps", bufs=2,
                                                 space="PSUM"))

            def load_wqk(wqk_d, rnd):
                wt = []
                for kc in range(KC):
                    t_ = wqk_pool.tile([128, 2 * CP], BF16,
                                       name=f"wqk{rnd}_{kc}", tag="wqk")
                    eng = nc.scalar if kc % 2 else nc.sync
                    eng.dma_start(t_[:], wqk_d[128 * kc:128 * (kc + 1), :])
                    wt.append(t_)
                return wt

            def load_wv(wv_d, rnd):
                wt = []
                for kc in range(KC):
                    t_ = wv_pool.tile([128, CP], BF16,
                                      name=f"wv{rnd}_{kc}", tag="wv")
                    eng = nc.scalar if kc % 2 else nc.sync
                    eng.dma_start(t_[:], wv_d[128 * kc:128 * (kc + 1), :])
                    wt.append(t_)
                return wt

            def qk_tiles(rnd):
                return [qk_pool.tile([128, T], BF16, name=f"qk{rnd}_{jc}",
                                     tag="qk") for jc in range(4)]

            def proj_qk_unit(wt, src, biasc, qk, jc, t4):
                """One [128,512] q^T/k^T block: 8 matmuls + bias add."""
                ps = cps.tile([128, 512], F32, tag="cps", name="ps_pqk")
                for kc in range(KC):
                    nc.tensor.matmul(
                        ps[:], wt[kc][:, 128 * jc:128 * (jc + 1)],
                        src[kc][:, 512 * t4:512 * (t4 + 1)],
                        start=(kc == 0), stop=(kc == KC - 1))
                nc.vector.tensor_scalar_add(
                    qk[jc][:, 512 * t4:512 * (t4 + 1)], ps[:],
                    biasc[:, jc:jc + 1])

            def proj_v_unit(wt, src, va, rnd, tb):
                """One 128-token block of v (natural layout + ones col)."""
                t_ = vaug_pool.tile([128, HG, DH + 1], BF16,
                                    name=f"va{rnd}_{tb}", tag="va")
                ps = cps.tile([128, 512], F32, tag="cps", name="ps_pv")
                for kc in range(KC):
                    nc.tensor.matmul(
                        ps[:, 0:CP], src[kc][:, 128 * tb:128 * (tb + 1)],
                        wt[kc][:], start=(kc == 0), stop=(kc == KC - 1))
                nc.vector.tensor_copy(
                    t_[:, :, 0:DH],
                    ps[:, 0:CP].rearrange("p (h d) -> p h d", h=HG))
                nc.vector.tensor_copy(t_[:, :, DH:DH + 1],
                                      ones_sb[:, :, None])
                va.append(t_)

            def attend_iter(qk, va, biasv, ob, q4, hh):
                """One (q4, hh) attention iteration, software-pipelined:
                scores+exp run one key block ahead of the AV matmuls."""
                nblk = 4 * (q4 + 1)
                qt, kt = qk[hh], qk[2 + hh]
                av = [avps.tile([DH + 1, 512], F32, tag="av",
                                name=f"av{h2}") for h2 in range(2)]
                pending = []

                def emit_scores(kb):
                    diag = kb - 4 * q4
                    c0 = max(0, 128 * diag)
                    npr = 512 - c0
                    ps = psum.tile([128, 2, 512], F32, tag="ps", name="s2")
                    for h2 in range(2):
                        base = 64 * h2
                        nc.tensor.matmul(
                            ps[:, h2, 0:npr],
                            kt[base:base + 64, 128 * kb:128 * (kb + 1)],
                            qt[base:base + 64,
                               512 * q4 + c0:512 * (q4 + 1)],
                            start=True, stop=True)
                    p = p_pool.tile([128, 2, 512], BF16, tag="p", name="p2")
                    nc.scalar.activation(p[:, :, 0:npr], ps[:, :, 0:npr],
                                         EXP, scale=SCALE)
                    if diag >= 0:
                        for h2 in range(2):
                            nc.vector.tensor_mul(
                                p[:, h2, 0:128], p[:, h2, 0:128], mask_sb[:])
                    pending.append((p, kb, c0, npr))

                def emit_av():
                    p, kb, c0, npr = pending.pop(0)
                    for h2 in range(2):
                        h = 2 * hh + h2
                        nc.tensor.matmul(
                            av[h2][:, c0:512], va[kb][:, h, :],
                            p[:, h2, 0:npr],
                            start=(kb == 0), stop=(kb == nblk - 1))

                for kb in range(nblk):
                    emit_scores(kb)
                    if len(pending) == 4:
                        emit_av()
                while pending:
                    emit_av()

                recr = sm_pool.tile([1, 1024], BF16, tag="recr", name="recr")
                with nc.allow_low_precision(
                        reason="softmax recip rounds to bf16"):
                    nc.vector.reciprocal(recr[0:1, 0:512],
                                         av[0][DH:DH + 1, :])
                    nc.vector.reciprocal(recr[0:1, 512:1024],
                                         av[1][DH:DH + 1, :])
                bps = cps.tile([128, 512], F32, tag="cps", name="bps")
                for h2 in range(2):
                    nc.tensor.matmul(
                        bps[64 * h2:64 * (h2 + 1), :], ones64_sb[:],
                        recr[0:1, 512 * h2:512 * (h2 + 1)],
                        start=True, stop=True)
                bc = bc_pool.tile([128, 512], F32, tag="bc", name="bc")
                nc.vector.tensor_copy(bc[:], bps[:])
                osl = ob[hh][:, 512 * q4:512 * (q4 + 1)]
                for h2 in range(2):
                    nc.vector.tensor_mul(
                        osl[64 * h2:64 * (h2 + 1), :],
                        av[h2][0:DH, :],
                        bc[64 * h2:64 * (h2 + 1), :])
                nc.vector.tensor_scalar_add(osl, osl, biasv[:, hh:hh + 1])

            def run_iters(iters, comps, after=None):
                """Emit attention iterations with companion units spread
                between them (fills the PE while the Act engine works)."""
                comps = list(comps)
                n = len(iters)
                for i, (fn, args) in enumerate(iters):
                    fn(*args)
                    left = n - i - 1
                    if comps:
                        take = (len(comps) if left == 0
                                else max(1, len(comps) // (left + 1)))
                        for _ in range(take):
                            if comps:
                                comps.pop(0)()
                    if after is not None:
                        after(i)
                for c in comps:
                    c()

            def emit_pred(half, ob):
                cs = slice(TH * half, TH * (half + 1))
                for hh in range(2):
                    nc.sync.dma_start(
                        pred_part[half][128 * hh:128 * (hh + 1), :],
                        ob[hh][:, cs])
                if sim:
                    nc.sync.dma_start(pred_full[half, 0:CP, :],
                                      pred_part[half][:, :])
                else:
                    nc.gpsimd.collective_compute(
                        "AllGather", mybir.AluOpType.bypass,
                        replica_groups=GROUPS,
                        ins=[pred_part[half][:, :]],
                        outs=[pred_full[half, :, :]])

            def emit_corr(hh, obh):
                for s in range(G):
                    for dup in range(2):
                        c_ = 4 * dup + s
                        nc.sync.dma_start(
                            corr_a2a[hh][128 * c_:128 * (c_ + 1), :],
                            obh[:, 512 * s:512 * (s + 1)])
                if sim:
                    nc.sync.dma_start(corr_out[hh][0:128, 0:64],
                                      corr_a2a[hh][0:128, 0:64])
                else:
                    nc.gpsimd.collective_compute(
                        "AllToAll", mybir.AluOpType.bypass,
                        replica_groups=GROUP8,
                        ins=[corr_a2a[hh][:, :]],
                        outs=[corr_out[hh][:, :]])

            def resid_unit(hf, kc):
                def f():
                    cs = slice(TH * hf, TH * (hf + 1))
                    pt = o_pool.tile([128, TH], BF16, tag="op", name="predld")
                    nc.sync.dma_start(
                        pt[:], pred_full[hf, 128 * kc:128 * (kc + 1), :])
                    nc.vector.tensor_sub(xt[kc][:, cs], xt[kc][:, cs], pt[:])
                return f

            # ================= phase A: round-0 projection ==============
            wv0_t = load_wv(wv0, 0)
            for hf in range(4):
                for kc in range(KC):
                    eng = nc.scalar if (kc + hf) % 2 else nc.sync
                    eng.dma_start(xt[kc][:, 512 * hf:512 * (hf + 1)],
                                  xT[128 * kc:128 * (kc + 1),
                                     512 * hf:512 * (hf + 1)])
                if hf == 0:
                    wqk0_t = load_wqk(wqk0, 0)
            va0 = []
            qk0 = qk_tiles(0)
            # emit only what attn0's first token half needs, the rest
            # becomes companion work under attention
            for tb in range(8):
                proj_v_unit(wv0_t, xt, va0, 0, tb)
            for t4 in range(2):
                for jc in range(4):
                    proj_qk_unit(wqk0_t, xt, bqk_sb[0], qk0, jc, t4)

            # previous rep's gate/output tail: emitted here so its corr
            # A2A latency and PE tail overlap this rep's projection start
            if pending_tail is not None:
                pending_tail(cps)
                pending_tail = None

            ob0 = [ob_pool.tile([128, T], BF16, tag="ob", name=f"ob0_{hh}")
                   for hh in range(2)]
            ob1 = [ob_pool.tile([128, T], BF16, tag="ob", name=f"ob1_{hh}")
                   for hh in range(2)]

            # ============ phase B: attn0 first token half ===============
            it0 = lambda q4, hh: (attend_iter, (qk0, va0, bv_sb[0], ob0,
                                                q4, hh))
            comps = [(lambda tb=tb: proj_v_unit(wv0_t, xt, va0, 0, tb))
                     for tb in range(8, 16)]
            for t4 in range(2, 4):
                comps += [(lambda jc=jc, t4=t4: proj_qk_unit(
                    wqk0_t, xt, bqk_sb[0], qk0, jc, t4)) for jc in range(4)]
            run_iters([it0(0, 0), it0(0, 1), it0(1, 0), it0(1, 1)], comps)
            emit_pred(0, ob0)

            # ============ phase C: attn0 second half + round-1 proj =====
            wv1_t = load_wv(wv1, 1)
            wqk1_t = load_wqk(wqk1, 1)
            va1 = []
            qk1 = qk_tiles(1)
            comps = [resid_unit(0, kc) for kc in range(KC)]
            comps += [(lambda tb=tb: proj_v_unit(wv1_t, xt, va1, 1, tb))
                      for tb in range(8)]
            for t4 in range(2):
                comps += [(lambda jc=jc, t4=t4: proj_qk_unit(
                    wqk1_t, xt, bqk_sb[1], qk1, jc, t4)) for jc in range(4)]
            run_iters([it0(2, 0), it0(2, 1), it0(3, 0), it0(3, 1)], comps)
            emit_pred(1, ob0)

            # ============ phase D: attn1 first half + rest of proj ======
            # prefetch gate-phase pred strips (local read of the AG output)
            pred_v = pred_full[:, :, :].rearrange("h d (s t) -> h d s t",
                                                  s=2)
            predg = []
            for cc in range(KC):
                pg_ = wqk_pool.tile([128, TS], BF16, name=f"predg{cc}",
                                    tag="wqk")
                nc.gpsimd.dma_start(
                    pg_[:], pred_v[bass.ds(hf_sel, 1),
                                   128 * cc:128 * (cc + 1),
                                   bass.ds(st_sel, 1), :]
                    .squeeze(2).squeeze(0))
                predg.append(pg_)

            it1 = lambda q4, hh: (attend_iter, (qk1, va1, bv_sb[1], ob1,
                                                q4, hh))
            comps = [resid_unit(1, kc) for kc in range(KC)]
            comps += [(lambda tb=tb: proj_v_unit(wv1_t, xt, va1, 1, tb))
                      for tb in range(8, 16)]
            for t4 in range(2, 4):
                comps += [(lambda jc=jc, t4=t4: proj_qk_unit(
                    wqk1_t, xt, bqk_sb[1], qk1, jc, t4)) for jc in range(4)]
            run_iters([it1(0, 0), it1(1, 0), it1(0, 1), it1(1, 1)], comps)

            # ============ phase E: attn1 second half + gate pred part ===
            gp_pre = [None] * KC

            def wg_slice(cc, jc):
                return wg_t[cc // 2][:, D * (cc % 2) + 128 * jc:
                                     D * (cc % 2) + 128 * (jc + 1)]

            def inject_gate(jc):
                def f():
                    ps = cps.tile([128, 512], F32, tag="cps", name="gp_ps")
                    for cc in range(KC):
                        nc.tensor.matmul(ps[:], wg_slice(cc, jc),
                                         predg[cc][:],
                                         start=(cc == 0), stop=(cc == KC - 1))
                    t_ = gpre_pool.tile([128, TS], BF16, name=f"gpre{jc}",
                                        tag="gpre")
                    nc.vector.tensor_copy(t_[:], ps[:])
                    gp_pre[jc] = t_
                return f

            comps = [inject_gate(jc) for jc in range(KC)]

            def after_e(i):
                if i == 1:
                    emit_corr(0, ob1[0])
                elif i == 3:
                    emit_corr(1, ob1[1])

            run_iters([it1(2, 0), it1(3, 0), it1(2, 1), it1(3, 1)], comps,
                      after=after_e)

        # ================= phase F: gate + output ==================
        def make_tail(wg_slice=wg_slice, gp_pre=gp_pre, predg=predg):
          def tail(cps_pool):
            gate_tail(nc, tc, sim, wg_slice, gp_pre, predg, wg_t, wo_t,
                      qk_pool, bg_sb, bsel_y, corr_out, y, cps_pool)
          return tail
        pending_tail = make_tail()
    if pending_tail is not None:
        pending_tail(None)


def gate_tail(nc, tc, sim, wg_slice, gp_pre, predg, wg_t, wo_t,
              qk_pool, bg_sb, bsel_y, corr_out, y, cps_pool):
    with ExitStack() as gat:
        gp_pool = gat.enter_context(tc.tile_pool(name="gp", bufs=8))
        y_pool = gat.enter_context(tc.tile_pool(name="yp", bufs=2))
        if cps_pool is None:
            cps_pool = gat.enter_context(tc.tile_pool(name="ps2", bufs=2,
                                                      space="PSUM"))

        # corr strip: channel 128cc belongs to group member cc//2, head
        # pair cc%2 — even chunks land with A2A half 0, odd with half 1;
        # consume in that order so the tail A2A overlaps the gate matmul
        corr_bv = [t[:, :].rearrange("(b r) t -> b r t", b=2)
                   for t in corr_out]
        corr_t = [qk_pool.tile([128, T], BF16, name=f"corrt{i}", tag="qk")
                  for i in range(2)]
        for cc in [0, 2, 4, 6, 1, 3, 5, 7]:
            r0 = 128 * (cc // 2)
            nc.sync.dma_start(
                corr_t[cc // 4][:, 512 * (cc % 4):512 * (cc % 4 + 1)],
                corr_bv[cc % 2][bass.ds(bsel_y, 1),
                                r0:r0 + 128, :].squeeze(0))
        corrg = [corr_t[cc // 4][:, 512 * (cc % 4):512 * (cc % 4 + 1)]
                 for cc in range(KC)]

        pgt = []
        for jc in range(KC):
            ps = cps_pool.tile([128, 512], F32, tag="cps", name=f"ps_g{jc}")
            for i, cc in enumerate([0, 2, 4, 6, 1, 3, 5, 7]):
                nc.tensor.matmul(ps[:], wg_slice(KC + cc, jc), corrg[cc],
                                 start=(i == 0), stop=(i == KC - 1))
            # fold in the pred-half partial computed during attention 1
            nc.vector.tensor_add(ps[:], ps[:], gp_pre[jc][:])
            gt = gp_pool.tile([128, TS], BF16, name=f"gate{jc}", tag="gp")
            nc.scalar.activation(gt[:], ps[:], SIG,
                                 bias=bg_sb[:, jc:jc + 1])
            nc.vector.tensor_mul(gt[:], gt[:], corrg[jc])
            nc.vector.tensor_add(gt[:], gt[:], predg[jc][:])
            pgt.append(gt)

        for tb in range(4):
            for n2 in range(2):
                yt = y_pool.tile([128, 512], F32, tag="y", name="yt")
                ps = cps_pool.tile([128, 512], F32, tag="cps", name="ps_y")
                for cc in range(KC):
                    nc.tensor.matmul(
                        ps[:], pgt[cc][:, 128 * tb:128 * (tb + 1)],
                        wo_t[cc][:, 512 * n2:512 * (n2 + 1)],
                        start=(cc == 0), stop=(cc == KC - 1))
                nc.vector.tensor_copy(yt[:], ps[:])
                nc.sync.dma_start(
                    y[128 * tb:128 * (tb + 1), 512 * n2:512 * (n2 + 1)],
                    yt[:])


_NC = None


def _get_nc():
    global _NC
    if _NC is None:
        _NC = _build()
    return _NC


def make_in_maps(x, Wqkv0, bqkv0, Wqkv1, bqkv1, Wg, bg, Wo, bo):
    bf = mybir.dt.np(BF16)
    mask_np = np.where(np.arange(128)[:, None] > np.arange(128)[None, :],
                       0.0, 1.0).astype(bf)
    ones_np = np.ones((128, HG), bf)
    ones64_np = np.ones((1, 64), bf)
    bg_a = np.ascontiguousarray(bg.reshape(D // 128, 128).T.astype(np.float32))
    wg_np = np.ascontiguousarray(Wg.astype(np.float32).astype(bf))
    wo_np = np.ascontiguousarray(Wo.astype(np.float32).astype(bf))

    in_maps = []
    for c in range(8):
        b, g = divmod(c, G)
        cq = slice(CP * g, CP * (g + 1))
        ck = slice(D + CP * g, D + CP * (g + 1))
        cv = slice(2 * D + CP * g, 2 * D + CP * (g + 1))
        m = {
            "xT": np.ascontiguousarray(x[b].T.astype(np.float32).astype(bf)),
            "mask01": mask_np, "onesc": ones_np, "bg": bg_a,
            "ones64": ones64_np,
            "wg": wg_np, "wo": wo_np,
        }
        for r, (W, bb) in enumerate(((Wqkv0, bqkv0), (Wqkv1, bqkv1))):
            m[f"wqk{r}"] = np.ascontiguousarray(
                np.concatenate([W[:, cq], W[:, ck]], axis=1)
                .astype(np.float32).astype(bf))
            m[f"wv{r}"] = np.ascontiguousarray(
                W[:, cv].astype(np.float32).astype(bf))
            bqk_cat = np.concatenate([bb[cq], bb[ck]]).astype(np.float32)
            m[f"bqk{r}"] = np.ascontiguousarray(bqk_cat.reshape(4, 128).T)
            m[f"bv{r}"] = np.ascontiguousarray(
                bb[cv].astype(np.float32).reshape(2, 128).T)
        in_maps.append(m)
    return in_maps


def assemble(results, bo):
    out = np.empty((B, T, D), np.float32)
    for c in range(8):
        b, g = divmod(c, G)
        out[b, TS * g:TS * (g + 1), :] = results[c]["y"]
    return out + bo.astype(np.float32)


def kernel(x, Wqkv0, bqkv0, Wqkv1, bqkv1, Wg, bg, Wo, bo):
    args = [np.asarray(a) for a in
            (x, Wqkv0, bqkv0, Wqkv1, bqkv1, Wg, bg, Wo, bo)]
    nc = _get_nc()
    in_maps = make_in_maps(*args)
    res = bass_utils.run_bass_kernel_spmd(nc, in_maps, core_ids=list(range(8)))
    return assemble(res.results, args[8])
